# revision 43
# baseline (speedup 1.0000x reference)
"""Multi-head attention (B=2, S=2048, D=1024, H=16) on 8 Trainium2 NeuronCores.

Sharding: core i handles batch b = i//4 and head-group hg = i%4 (4 heads).
The fc layer is sharded over its contraction dim (each core emits a partial
y summed on the host); Wv/bv are folded into Wfc/bfc on the host (exact).

Algorithm (v3):
  - Wq is folded into the K side on the host: score = q . k~ with
    k~ = (Wq^T Wk / sqrt(hd)) k + bias-row, so no on-device projections.
    log2(e) and a power-of-two fp8 range scale are folded in as well, so
    the device computes t = score*log2e and exponentiates as 2^t.
  - scores: fp8e4m3 DoubleRow matmuls (contraction 2x33 packs the 64 head
    channels + bias row), out [128 keys, 512 q] psum f32 at 0.5 cycles/row.
  - exp: split per k-tile across the only two PSUM-capable elementwise
    engines (GPSIMD cannot access PSUM on TRN2): ScalarE exact exp->bf16
    (scale=ln2/ascale) for 10/16 tiles, DVE Schraudolph for 6/16 (one
    tensor_scalar mult+add writing int16 exponent-bits through a
    bf16-tile bitcast, ~3% rel err; end-to-end rel err ~1.2e-2).
  - AV transposed: exp tile is the stationary operand [128 keys, 128 q],
    V (bf16, with a ones column for the denominator) streams as moving
    [128, 65] -> out [q, 64ch|den] psum, 65 cycles per k-tile: softmax
    normalization becomes a per-partition reciprocal+scalar-multiply.
  - oTn [128 q, 64A|64B] bf16 pairs are PE-transposed (identity moving)
    to [128 ch, 128 q] and the fc runs K=256 over two bf16 matmuls per
    512-wide psum bank; y is evacuated to SBUF (DVE) and DMA'd out.

Schedule: one flat software-pipelined stream over the 8 (q-window, pair)
units; scores/exp run 3 k-tile steps ahead of the AV consumers. All psum
transients (score tiles, transpose psum, fc psum) share one 3-slot
[128,1024] rotation (6 banks) + 2 banks of AV accumulators; a start=True
matmul zeroes its whole 2KB bank, so interleaved accumulation groups in
one bank carry exactly one start. Tail work (normalize, transpose, oT
evac, fc) is deferred into later stream steps via a not-before-gi queue
so it never blocks the in-order ScalarE/DVE queues at a pair boundary.
"""


import sys

import numpy as np

if "/opt/trn_rl_repo" not in sys.path:
    sys.path.insert(0, "/opt/trn_rl_repo")

HEAD = 16
B, S, D = 2, 2048, 1024
HD = 64
HPC = 4          # heads per core
CH = HPC * HD    # channels per core
N_CORES = 8
NKT = S // 128   # k tiles
NQB = S // 512   # q windows
LOG2E = 1.4426950408889634
ASCALE = 2.0     # fp8 range scale folded into k~; undone in the exp scale

_CACHE = {}
LAST_RESULTS = None


# Per-pair k-tile positions whose exp runs on DVE (Schraudolph); the rest
# run exact exp on ScalarE. 6/16 on DVE globally balances ScalarE's 1038ns
# exp + oT-evac load against DVE's 1193ns exp + y-evac/norm/recip load,
# and keeping kt 0/1 on ScalarE lets DVE run the previous pair's deferred
# normalization immediately at each pair boundary.
# GPSIMD can't touch PSUM on TRN2, so only these two engines qualify.
_DVE_KTS = {1, 4, 6, 9, 11, 14}


def _build():
    import concourse.tile as tile
    from concourse import bacc, mybir

    f32 = mybir.dt.float32
    bf16 = mybir.dt.bfloat16
    fp8 = mybir.dt.float8e4
    i16 = mybir.dt.int16
    EXP = mybir.ActivationFunctionType.Exp
    DR = mybir.MatmulPerfMode.DoubleRow
    MULT = mybir.AluOpType.mult
    ADD = mybir.AluOpType.add

    nc = bacc.Bacc("TRN2", target_bir_lowering=False, debug=False,
                   num_devices=N_CORES)

    # unused internal tensor whose name varies per retry: changes the BIR
    # content hash so a retry never reuses a possibly-corrupt cached NEFF
    nonce = _CACHE.get("nonce", 0)
    if nonce:
        nc.dram_tensor(f"retry_nonce_{nonce}", [1, 1], mybir.dt.float32)

    qt_d = nc.dram_tensor("qtil", [33, 2 * HPC, S], fp8, kind="ExternalInput")
    kt_d = nc.dram_tensor("ktil", [33, 2 * HPC, S], fp8, kind="ExternalInput")
    v1_d = nc.dram_tensor("v1", [128, NKT, HPC, 65], bf16,
                          kind="ExternalInput")
    wf_d = nc.dram_tensor("wf", [128, 2, D], bf16, kind="ExternalInput")
    id_d = nc.dram_tensor("iden", [128, 128], bf16, kind="ExternalInput")
    qk0_d = nc.dram_tensor("qk0", [33, 8, 512], fp8, kind="ExternalInput")
    y_d = nc.dram_tensor("y", [S, D], f32, kind="ExternalOutput")

    act_scale = float(np.log(2.0) / ASCALE)
    sch_mul = float(128.0 / ASCALE)
    sch_add = float(127 * 128 - 5.5)

    with tile.TileContext(nc) as tc, nc.allow_low_precision(
            reason="bf16/fp8 operands feed f32-psum matmuls"):
        with (
            tc.tile_pool(name="inp", bufs=1) as inp,
            tc.tile_pool(name="expp", bufs=8) as expp,
            tc.tile_pool(name="otnp", bufs=8) as otnp,
            tc.tile_pool(name="otp", bufs=1) as otp,
            tc.tile_pool(name="recp", bufs=4) as recp,
            tc.tile_pool(name="ysbp", bufs=3) as ysbp,
            tc.tile_pool(name="ps_big", bufs=3, space="PSUM") as ps_big,
            tc.tile_pool(name="ps_av", bufs=2, space="PSUM") as ps_av,
        ):
            # ------------- inputs (chunked; critical path first) ----------
            qtil = inp.tile([33, 2 * HPC, S], fp8, tag="qtil")
            ktil = inp.tile([33, 2 * HPC, S], fp8, tag="ktil")
            v1 = inp.tile([128, NKT, HPC, 65], bf16, tag="v1")
            wf = inp.tile([128, 2, D], bf16, tag="wf")
            iden = inp.tile([128, 128], bf16, tag="iden")

            # one combined first-chunk DMA (k+q of head-pair 0, first 512
            # keys/queries) so a single transfer gates the first matmul
            qk0 = inp.tile([33, 8, 512], fp8, tag="qk0")
            nc.sync.dma_start(out=qk0, in_=qk0_d[:, :, :])
            nc.sync.dma_start(out=v1[:, 0:4, :, :], in_=v1_d[:, 0:4, :, :])
            nc.sync.dma_start(out=ktil[:, 0:4, 0:512], in_=kt_d[:, 0:4, 0:512])
            nc.sync.dma_start(out=ktil[:, 0:4, 512:2048],
                              in_=kt_d[:, 0:4, 512:2048])
            nc.sync.dma_start(out=qtil[:, 0:4, 0:512], in_=qt_d[:, 0:4, 0:512])
            nc.sync.dma_start(out=iden, in_=id_d[:, :])
            for c in range(1, 8):
                nc.sync.dma_start(out=v1[:, 2 * c:2 * c + 2, :, :],
                                  in_=v1_d[:, 2 * c:2 * c + 2, :, :])
            nc.sync.dma_start(out=ktil[:, 4:8, :], in_=kt_d[:, 4:8, :])
            nc.sync.dma_start(out=qtil[:, 4:8, 0:512], in_=qt_d[:, 4:8, 0:512])
            nc.sync.dma_start(out=qtil[:, :, 512:2048],
                              in_=qt_d[:, :, 512:2048])
            nc.sync.dma_start(out=wf, in_=wf_d[:, :, :])

            oT = []
            for p in range(2):
                t = otp.tile([128, S], bf16, tag=f"oT{p}", name=f"oT{p}")
                oT.append(t)

            # tail work (normalize/transpose/evac/fc) is deferred into the
            # NEXT pair's k-tile stream via a pop queue so it never blocks
            # the in-order Act/DVE queues at a pair boundary
            deferred = []

            def emit_norm(av, rec, otn, h2, on_act=False):
                nc.vector.reciprocal(out=rec, in_=av[:, 64:512:128])
                for qs in range(4):
                    if on_act:
                        nc.scalar.mul(otn[qs][:, 64 * h2:64 * h2 + 64],
                                      av[:, 128 * qs:128 * qs + 64],
                                      rec[:, qs:qs + 1])
                    else:
                        nc.vector.tensor_scalar(
                            out=otn[qs][:, 64 * h2:64 * h2 + 64],
                            in0=av[:, 128 * qs:128 * qs + 64],
                            scalar1=rec[:, qs:qs + 1], scalar2=None,
                            op0=MULT)

            def emit_transpose(otn, p, q0, qb):
                tp = ps_big.tile([128, 1024], f32, tag="sc",
                                 name=f"tp{qb}_{p}")[:, 0:256].bitcast(bf16)
                for qs in range(4):
                    nc.tensor.matmul(tp[:, 128 * qs:128 * qs + 128],
                                     otn[qs], iden[:, :], is_transpose=True,
                                     start=qs == 0, stop=True,
                                     skip_group_check=True)
                return tp

            def emit_ot_evac(tp, p, q0):
                nc.scalar.copy(oT[p][:, q0:q0 + 512], tp)

            def emit_fc_mm(stt):
                # two cb halves: a single matmul may not write across a
                # 2KB psum bank boundary, so the [128,1024] slot is filled
                # by two [128,512] accumulation groups
                yp = ps_big.tile([128, 1024], f32, tag="sc",
                                 name=f"y{stt}")
                for cb in range(2):
                    cs = slice(512 * cb, 512 * cb + 512)
                    nc.tensor.matmul(yp[:, cs],
                                     oT[0][:, 128 * stt:128 * stt + 128],
                                     wf[:, 0, cs], start=True, stop=False,
                                     skip_group_check=True)
                    nc.tensor.matmul(yp[:, cs],
                                     oT[1][:, 128 * stt:128 * stt + 128],
                                     wf[:, 1, cs], start=False, stop=True,
                                     skip_group_check=True)
                return yp

            def emit_fc_out(yp, stt, on_act=False):
                ysb = ysbp.tile([128, 1024], f32, tag="ysb",
                                name=f"ysb{stt}")
                if on_act:
                    nc.scalar.copy(ysb, yp)
                else:
                    nc.vector.tensor_copy(ysb, yp)
                nc.sync.dma_start(
                    out=y_d[128 * stt:128 * stt + 128, :], in_=ysb)

            # prewarm the Exp activation table during the input DMAs
            warm = recp.tile([128, 1], f32, tag="warm")
            nc.vector.memset(warm, 0.0)
            nc.scalar.activation(out=warm, in_=warm, func=EXP,
                                 scale=act_scale)

            # single flat software-pipelined stream over all 8 (qb, p)
            # pairs: scores/exp run 2 k-tile steps ahead of the AV
            # consumers, crossing pair boundaries without a bubble
            pairs = [(qb, p) for qb in range(NQB) for p in range(2)]
            NP = len(pairs)
            pend = []
            avt = {}
            for gi in range(NP * NKT + 3):
                while deferred and deferred[0][0] <= gi:
                    deferred.pop(0)[1]()
                if gi < NP * NKT:
                    pi, kt = divmod(gi, NKT)
                    qb, p = pairs[pi]
                    q0 = 512 * qb
                    hA, hB = 2 * p, 2 * p + 1
                    ks = slice(128 * kt, 128 * kt + 128)
                    sc = ps_big.tile([128, 1024], f32, tag="sc",
                                    name=f"sc{qb}_{p}_{kt}")
                    if pi == 0 and kt < 4:
                        kA, qA = qk0[:, 0:2, ks], qk0[:, 4:6, :]
                        kB, qB = qk0[:, 2:4, ks], qk0[:, 6:8, :]
                    else:
                        kA = ktil[:, 2 * hA:2 * hA + 2, ks]
                        qA = qtil[:, 2 * hA:2 * hA + 2, q0:q0 + 512]
                        kB = ktil[:, 2 * hB:2 * hB + 2, ks]
                        qB = qtil[:, 2 * hB:2 * hB + 2, q0:q0 + 512]
                    nc.tensor.matmul(sc[:, 0:512], kA, qA,
                                     start=True, stop=True, perf_mode=DR)
                    nc.tensor.matmul(sc[:, 512:1024], kB, qB,
                                     start=True, stop=True, perf_mode=DR)
                    ex = expp.tile([128, 1024], bf16, tag="ex",
                                   name=f"ex{qb}_{p}_{kt}")
                    if kt not in _DVE_KTS:
                        nc.scalar.activation(out=ex, in_=sc, func=EXP,
                                             scale=act_scale)
                    else:
                        nc.vector.tensor_scalar(
                            out=ex.bitcast(i16), in0=sc,
                            scalar1=sch_mul, scalar2=sch_add,
                            op0=MULT, op1=ADD)
                    pend.append((pi, kt, ex))
                if gi >= 3:
                    api, akt, aex = pend[gi - 3]
                    aqb, ap = pairs[api]
                    if akt == 0:
                        # one 2KB bank per accumulator; qs blocks padded to
                        # 128 floats so only the qs==0 matmul carries
                        # start=True (a start marks the whole 2KB
                        # zero-region, so interleaved groups must share a
                        # single start per bank)
                        avt[api] = (
                            ps_av.tile([128, 512], f32, tag="av",
                                       name=f"avA{api}"),
                            ps_av.tile([128, 512], f32, tag="av",
                                       name=f"avB{api}"))
                    st = akt == 0
                    sp = akt == NKT - 1
                    for h2, hh in ((0, 2 * ap), (1, 2 * ap + 1)):
                        av = avt[api][h2]
                        for qs in range(4):
                            nc.tensor.matmul(
                                av[:, 128 * qs:128 * qs + 65],
                                aex[:, 512 * h2 + 128 * qs:
                                    512 * h2 + 128 * qs + 128],
                                v1[:, akt, hh, :],
                                start=st and qs == 0, stop=sp,
                                skip_group_check=True)
                    if sp:
                        # pair finished: queue its tail work
                        avA, avB = avt.pop(api)
                        aq0 = 512 * aqb
                        otn = []
                        for qs in range(4):
                            t = otnp.tile([128, 128], bf16, tag="otn",
                                          name=f"otn{api}_{qs}")
                            otn.append(t)
                        recA = recp.tile([128, 4], f32, tag="rec",
                                         name=f"recA{api}")
                        recB = recp.tile([128, 4], f32, tag="rec",
                                         name=f"recB{api}")
                        last = api == NP - 1
                        deferred.append((gi,
                            lambda av=avA, r=recA, o=otn, la=last:
                                emit_norm(av, r, o, 0, on_act=la)))
                        deferred.append((gi + 1,
                            lambda av=avB, r=recB, o=otn:
                                emit_norm(av, r, o, 1, on_act=False)))
                        tpbox = []
                        deferred.append((gi + 2,
                            lambda o=otn, p=ap, q0=aq0, qb=aqb, b=tpbox:
                                b.append(emit_transpose(o, p, q0, qb))))
                        deferred.append((gi + 6,
                            lambda p=ap, q0=aq0, b=tpbox:
                                emit_ot_evac(b[0], p, q0)))
                        if ap == 1:
                            for i4 in range(4):
                                oa = last and i4 % 2 == 0
                                ybox = []
                                deferred.append((gi + 4 + 4 * i4,
                                    lambda stt=4 * aqb + i4, b=ybox:
                                        b.append(emit_fc_mm(stt))))
                                deferred.append((gi + 6 + 4 * i4,
                                    lambda stt=4 * aqb + i4, oa=oa, b=ybox:
                                        emit_fc_out(b[0], stt, on_act=oa)))
            while deferred:
                deferred.pop(0)[1]()

    nc.compile()
    return nc


def _prep(query, key, value, Wq, bq, Wk, bk, Wv, bv, Wfc, bfc):
    """Host-side sharding / layout prep. Returns (in_maps, bfc_eff)."""
    import ml_dtypes

    query = np.asarray(query, dtype=np.float32)
    key = np.asarray(key, dtype=np.float32)
    value = np.asarray(value, dtype=np.float32)
    Wq = np.asarray(Wq, np.float32); bq = np.asarray(bq, np.float32)
    Wk = np.asarray(Wk, np.float32); bk = np.asarray(bk, np.float32)
    Wv = np.asarray(Wv, np.float32); bv = np.asarray(bv, np.float32)
    Wfc = np.asarray(Wfc, np.float32); bfc = np.asarray(bfc, np.float32)

    s_hd = np.float32(1.0 / np.sqrt(HD))
    # fold Wq into the K side: score*log2e = q . (M k) + w . k   (per head)
    M = (Wq.T @ Wk) * (s_hd * LOG2E * ASCALE)          # [d, e]
    w_row = (bq @ Wk) * (s_hd * LOG2E * ASCALE)        # [e]

    # fold Wv / bv into fc
    A = np.empty((D, D), np.float32)
    bfc_eff = bfc.copy()
    for h in range(HEAD):
        Wfc_h = Wfc[:, HD * h:HD * h + HD]
        A[:, HD * h:HD * h + HD] = Wfc_h @ Wv
        bfc_eff += Wfc_h @ bv
    At = np.ascontiguousarray(A.T)                     # [ch, c]

    iden = np.eye(128, dtype=ml_dtypes.bfloat16)

    in_maps = []
    for core in range(N_CORES):
        b, hg = core // 4, core % 4
        ch0 = CH * hg
        qtil = np.zeros((33, 2 * HPC, S), np.float32)
        ktil = np.zeros((33, 2 * HPC, S), np.float32)
        v1 = np.empty((128, NKT, HPC, 65), np.float32)
        for h in range(HPC):
            qh = query[b][:, ch0 + HD * h:ch0 + HD * h + HD]   # [S, 64]
            kh = key[b][:, ch0 + HD * h:ch0 + HD * h + HD]
            kt = kh @ M.T                                      # [S, 64]
            qtil[0:32, 2 * h, :] = qh[:, 0:32].T
            qtil[0:32, 2 * h + 1, :] = qh[:, 32:64].T
            qtil[32, 2 * h, :] = 1.0
            ktil[0:32, 2 * h, :] = kt[:, 0:32].T
            ktil[0:32, 2 * h + 1, :] = kt[:, 32:64].T
            ktil[32, 2 * h, :] = kh @ w_row
            vh = value[b][:, ch0 + HD * h:ch0 + HD * h + HD]
            v1[:, :, h, 0:64] = vh.reshape(NKT, 128, HD).transpose(1, 0, 2)
            v1[:, :, h, 64] = 1.0
        wfc = np.empty((128, 2, D), np.float32)
        wfc[:, 0, :] = At[ch0:ch0 + 128]
        wfc[:, 1, :] = At[ch0 + 128:ch0 + 256]
        qk0 = np.concatenate([ktil[:, 0:4, 0:512], qtil[:, 0:4, 0:512]],
                             axis=1)
        in_maps.append({
            "qk0": qk0.astype(ml_dtypes.float8_e4m3).view(np.uint8),
            "qtil": qtil.astype(ml_dtypes.float8_e4m3).view(np.uint8),
            "ktil": ktil.astype(ml_dtypes.float8_e4m3).view(np.uint8),
            "v1": v1.astype(ml_dtypes.bfloat16).view(np.uint16),
            "wf": wfc.astype(ml_dtypes.bfloat16).view(np.uint16),
            "iden": iden.view(np.uint16),
        })
    return in_maps, bfc_eff


def _run_once(inputs):
    global LAST_RESULTS
    from concourse.bass_utils import run_bass_kernel_spmd

    if "nc" not in _CACHE:
        _CACHE["nc"] = _build()
    nc = _CACHE["nc"]

    in_maps, bfc_eff = _prep(**inputs)
    res = run_bass_kernel_spmd(nc, in_maps, core_ids=list(range(N_CORES)))
    LAST_RESULTS = res

    out = np.empty((B, S, D), np.float32)
    for b in range(B):
        acc = res.results[4 * b]["y"].astype(np.float32).copy()
        for hg in range(1, 4):
            acc += res.results[4 * b + hg]["y"]
        out[b] = acc + bfc_eff
    return out


def kernel(**inputs) -> np.ndarray:
    last_exc = None
    for attempt in range(3):
        try:
            out = _run_once(inputs)
            amax = float(np.abs(out).max())
            if np.isfinite(out).all() and 1e-6 < amax < 1e3:
                return out
            raise RuntimeError(f"implausible kernel output (absmax={amax})")
        except Exception as e:  # noqa: BLE001 - retry transient HW failures
            last_exc = e
            _CACHE.pop("nc", None)
            _CACHE["nonce"] = attempt + 1
    raise last_exc


# revision 44
# speedup vs baseline: 1.0059x; 1.0059x over previous
"""Multi-head attention (B=2, S=2048, D=1024, H=16) on 8 Trainium2 NeuronCores.

Sharding: core i handles batch b = i//4 and head-group hg = i%4 (4 heads).
The fc layer is sharded over its contraction dim (each core emits a partial
y summed on the host); Wv/bv are folded into Wfc/bfc on the host (exact).

Algorithm (v3):
  - Wq is folded into the K side on the host: score = q . k~ with
    k~ = (Wq^T Wk / sqrt(hd)) k + bias-row, so no on-device projections.
    log2(e) and a power-of-two fp8 range scale are folded in as well, so
    the device computes t = score*log2e and exponentiates as 2^t.
  - scores: fp8e4m3 DoubleRow matmuls (contraction 2x33 packs the 64 head
    channels + bias row), out [128 keys, 512 q] psum f32 at 0.5 cycles/row.
  - exp: split per k-tile across the only two PSUM-capable elementwise
    engines (GPSIMD cannot access PSUM on TRN2): ScalarE exact exp->bf16
    (scale=ln2/ascale) for 10/16 tiles, DVE Schraudolph for 6/16 (one
    tensor_scalar mult+add writing int16 exponent-bits through a
    bf16-tile bitcast, ~3% rel err; end-to-end rel err ~1.2e-2).
  - AV transposed: exp tile is the stationary operand [128 keys, 128 q],
    V (bf16, with a ones column for the denominator) streams as moving
    [128, 65] -> out [q, 64ch|den] psum, 65 cycles per k-tile: softmax
    normalization becomes a per-partition reciprocal+scalar-multiply.
  - oTn [128 q, 64A|64B] bf16 pairs are PE-transposed (identity moving)
    to [128 ch, 128 q] and the fc runs K=256 over two bf16 matmuls per
    512-wide psum bank; y is evacuated to SBUF (DVE) and DMA'd out.

Schedule: one flat software-pipelined stream over the 8 (q-window, pair)
units; scores/exp run 3 k-tile steps ahead of the AV consumers. All psum
transients (score tiles, transpose psum, fc psum) share one 3-slot
[128,1024] rotation (6 banks) + 2 banks of AV accumulators; a start=True
matmul zeroes its whole 2KB bank, so interleaved accumulation groups in
one bank carry exactly one start. Tail work (normalize, transpose, oT
evac, fc) is deferred into later stream steps via a not-before-gi queue
so it never blocks the in-order ScalarE/DVE queues at a pair boundary.
"""


import sys

import numpy as np

if "/opt/trn_rl_repo" not in sys.path:
    sys.path.insert(0, "/opt/trn_rl_repo")

HEAD = 16
B, S, D = 2, 2048, 1024
HD = 64
HPC = 4          # heads per core
CH = HPC * HD    # channels per core
N_CORES = 8
NKT = S // 128   # k tiles
NQB = S // 512   # q windows
LOG2E = 1.4426950408889634
ASCALE = 2.0     # fp8 range scale folded into k~; undone in the exp scale

_CACHE = {}
LAST_RESULTS = None


# Per-pair k-tile positions whose exp runs on DVE (Schraudolph); the rest
# run exact exp on ScalarE. 6/16 on DVE globally balances ScalarE's 1038ns
# exp + oT-evac load against DVE's 1193ns exp + y-evac/norm/recip load,
# and keeping kt 0/1 on ScalarE lets DVE run the previous pair's deferred
# normalization immediately at each pair boundary.
# GPSIMD can't touch PSUM on TRN2, so only these two engines qualify.
_DVE_KTS = {1, 4, 6, 9, 11, 14}


def _build():
    import concourse.tile as tile
    from concourse import bacc, mybir

    f32 = mybir.dt.float32
    bf16 = mybir.dt.bfloat16
    fp8 = mybir.dt.float8e4
    i16 = mybir.dt.int16
    EXP = mybir.ActivationFunctionType.Exp
    DR = mybir.MatmulPerfMode.DoubleRow
    MULT = mybir.AluOpType.mult
    ADD = mybir.AluOpType.add

    nc = bacc.Bacc("TRN2", target_bir_lowering=False, debug=False,
                   num_devices=N_CORES)

    # unused internal tensor whose name varies per retry: changes the BIR
    # content hash so a retry never reuses a possibly-corrupt cached NEFF
    nonce = _CACHE.get("nonce", 0)
    if nonce:
        nc.dram_tensor(f"retry_nonce_{nonce}", [1, 1], mybir.dt.float32)

    qt_d = nc.dram_tensor("qtil", [33, 2 * HPC, S], fp8, kind="ExternalInput")
    kt_d = nc.dram_tensor("ktil", [33, 2 * HPC, S], fp8, kind="ExternalInput")
    v1_d = nc.dram_tensor("v1", [128, NKT, HPC, 65], bf16,
                          kind="ExternalInput")
    wf_d = nc.dram_tensor("wf", [128, 2, D], bf16, kind="ExternalInput")
    id_d = nc.dram_tensor("iden", [128, 128], bf16, kind="ExternalInput")
    qk0_d = nc.dram_tensor("qk0", [33, 8, 512], fp8, kind="ExternalInput")
    y_d = nc.dram_tensor("y", [S, D], f32, kind="ExternalOutput")

    act_scale = float(np.log(2.0) / ASCALE)
    sch_mul = float(128.0 / ASCALE)
    sch_add = float(127 * 128 - 5.5)

    with tile.TileContext(nc) as tc, nc.allow_low_precision(
            reason="bf16/fp8 operands feed f32-psum matmuls"):
        with (
            tc.tile_pool(name="inp", bufs=1) as inp,
            tc.tile_pool(name="expp", bufs=8) as expp,
            tc.tile_pool(name="otnp", bufs=8) as otnp,
            tc.tile_pool(name="otp", bufs=1) as otp,
            tc.tile_pool(name="recp", bufs=4) as recp,
            tc.tile_pool(name="ysbp", bufs=4) as ysbp,
            tc.tile_pool(name="ps_big", bufs=3, space="PSUM") as ps_big,
            tc.tile_pool(name="ps_av", bufs=2, space="PSUM") as ps_av,
        ):
            # ------------- inputs (chunked; critical path first) ----------
            qtil = inp.tile([33, 2 * HPC, S], fp8, tag="qtil")
            ktil = inp.tile([33, 2 * HPC, S], fp8, tag="ktil")
            v1 = inp.tile([128, NKT, HPC, 65], bf16, tag="v1")
            wf = inp.tile([128, 2, D], bf16, tag="wf")
            iden = inp.tile([128, 128], bf16, tag="iden")

            # one combined first-chunk DMA (k+q of head-pair 0, first 512
            # keys/queries) so a single transfer gates the first matmul
            qk0 = inp.tile([33, 8, 512], fp8, tag="qk0")
            nc.sync.dma_start(out=qk0, in_=qk0_d[:, :, :])
            nc.sync.dma_start(out=v1[:, 0:4, :, :], in_=v1_d[:, 0:4, :, :])
            nc.sync.dma_start(out=ktil[:, 0:4, 0:512], in_=kt_d[:, 0:4, 0:512])
            nc.sync.dma_start(out=ktil[:, 0:4, 512:2048],
                              in_=kt_d[:, 0:4, 512:2048])
            nc.sync.dma_start(out=qtil[:, 0:4, 0:512], in_=qt_d[:, 0:4, 0:512])
            nc.sync.dma_start(out=iden, in_=id_d[:, :])
            for c in range(1, 8):
                nc.sync.dma_start(out=v1[:, 2 * c:2 * c + 2, :, :],
                                  in_=v1_d[:, 2 * c:2 * c + 2, :, :])
            nc.sync.dma_start(out=ktil[:, 4:8, :], in_=kt_d[:, 4:8, :])
            nc.sync.dma_start(out=qtil[:, 4:8, 0:512], in_=qt_d[:, 4:8, 0:512])
            nc.sync.dma_start(out=qtil[:, :, 512:2048],
                              in_=qt_d[:, :, 512:2048])
            nc.sync.dma_start(out=wf, in_=wf_d[:, :, :])

            oT = []
            for p in range(2):
                t = otp.tile([128, S], bf16, tag=f"oT{p}", name=f"oT{p}")
                oT.append(t)

            # tail work (normalize/transpose/evac/fc) is deferred into the
            # NEXT pair's k-tile stream via a pop queue so it never blocks
            # the in-order Act/DVE queues at a pair boundary
            deferred = []

            def emit_norm(av, rec, otn, h2, on_act=False):
                nc.vector.reciprocal(out=rec, in_=av[:, 64:512:128])
                for qs in range(4):
                    if on_act:
                        nc.scalar.mul(otn[qs][:, 64 * h2:64 * h2 + 64],
                                      av[:, 128 * qs:128 * qs + 64],
                                      rec[:, qs:qs + 1])
                    else:
                        nc.vector.tensor_scalar(
                            out=otn[qs][:, 64 * h2:64 * h2 + 64],
                            in0=av[:, 128 * qs:128 * qs + 64],
                            scalar1=rec[:, qs:qs + 1], scalar2=None,
                            op0=MULT)

            def emit_transpose(otn, p, q0, qb):
                tp = ps_big.tile([128, 1024], f32, tag="sc",
                                 name=f"tp{qb}_{p}")[:, 0:256].bitcast(bf16)
                for qs in range(4):
                    nc.tensor.matmul(tp[:, 128 * qs:128 * qs + 128],
                                     otn[qs], iden[:, :], is_transpose=True,
                                     start=qs == 0, stop=True,
                                     skip_group_check=True)
                return tp

            def emit_ot_evac(tp, p, q0):
                nc.scalar.copy(oT[p][:, q0:q0 + 512], tp)

            def emit_fc_mm(stt):
                # two cb halves: a single matmul may not write across a
                # 2KB psum bank boundary, so the [128,1024] slot is filled
                # by two [128,512] accumulation groups
                yp = ps_big.tile([128, 1024], f32, tag="sc",
                                 name=f"y{stt}")
                for cb in range(2):
                    cs = slice(512 * cb, 512 * cb + 512)
                    nc.tensor.matmul(yp[:, cs],
                                     oT[0][:, 128 * stt:128 * stt + 128],
                                     wf[:, 0, cs], start=True, stop=False,
                                     skip_group_check=True)
                    nc.tensor.matmul(yp[:, cs],
                                     oT[1][:, 128 * stt:128 * stt + 128],
                                     wf[:, 1, cs], start=False, stop=True,
                                     skip_group_check=True)
                return yp

            def emit_fc_out(yp, stt, on_act=False):
                ysb = ysbp.tile([128, 1024], f32, tag="ysb",
                                name=f"ysb{stt}")
                if on_act:
                    nc.scalar.copy(ysb, yp)
                else:
                    nc.vector.tensor_copy(ysb, yp)
                nc.sync.dma_start(
                    out=y_d[128 * stt:128 * stt + 128, :], in_=ysb)

            # prewarm the Exp activation table during the input DMAs
            warm = recp.tile([128, 1], f32, tag="warm")
            nc.vector.memset(warm, 0.0)
            nc.scalar.activation(out=warm, in_=warm, func=EXP,
                                 scale=act_scale)

            # single flat software-pipelined stream over all 8 (qb, p)
            # pairs: scores/exp run 2 k-tile steps ahead of the AV
            # consumers, crossing pair boundaries without a bubble
            pairs = [(qb, p) for qb in range(NQB) for p in range(2)]
            NP = len(pairs)
            pend = []
            avt = {}
            for gi in range(NP * NKT + 3):
                while deferred and deferred[0][0] <= gi:
                    deferred.pop(0)[1]()
                if gi < NP * NKT:
                    pi, kt = divmod(gi, NKT)
                    qb, p = pairs[pi]
                    q0 = 512 * qb
                    hA, hB = 2 * p, 2 * p + 1
                    ks = slice(128 * kt, 128 * kt + 128)
                    sc = ps_big.tile([128, 1024], f32, tag="sc",
                                    name=f"sc{qb}_{p}_{kt}")
                    if pi == 0 and kt < 4:
                        kA, qA = qk0[:, 0:2, ks], qk0[:, 4:6, :]
                        kB, qB = qk0[:, 2:4, ks], qk0[:, 6:8, :]
                    else:
                        kA = ktil[:, 2 * hA:2 * hA + 2, ks]
                        qA = qtil[:, 2 * hA:2 * hA + 2, q0:q0 + 512]
                        kB = ktil[:, 2 * hB:2 * hB + 2, ks]
                        qB = qtil[:, 2 * hB:2 * hB + 2, q0:q0 + 512]
                    nc.tensor.matmul(sc[:, 0:512], kA, qA,
                                     start=True, stop=True, perf_mode=DR)
                    nc.tensor.matmul(sc[:, 512:1024], kB, qB,
                                     start=True, stop=True, perf_mode=DR)
                    ex = expp.tile([128, 1024], bf16, tag="ex",
                                   name=f"ex{qb}_{p}_{kt}")
                    if kt not in _DVE_KTS:
                        nc.scalar.activation(out=ex, in_=sc, func=EXP,
                                             scale=act_scale)
                    else:
                        nc.vector.tensor_scalar(
                            out=ex.bitcast(i16), in0=sc,
                            scalar1=sch_mul, scalar2=sch_add,
                            op0=MULT, op1=ADD)
                    pend.append((pi, kt, ex))
                if gi >= 3:
                    api, akt, aex = pend[gi - 3]
                    aqb, ap = pairs[api]
                    if akt == 0:
                        # one 2KB bank per accumulator; qs blocks padded to
                        # 128 floats so only the qs==0 matmul carries
                        # start=True (a start marks the whole 2KB
                        # zero-region, so interleaved groups must share a
                        # single start per bank)
                        avt[api] = (
                            ps_av.tile([128, 512], f32, tag="av",
                                       name=f"avA{api}"),
                            ps_av.tile([128, 512], f32, tag="av",
                                       name=f"avB{api}"))
                    st = akt == 0
                    sp = akt == NKT - 1
                    for h2, hh in ((0, 2 * ap), (1, 2 * ap + 1)):
                        av = avt[api][h2]
                        for qs in range(4):
                            nc.tensor.matmul(
                                av[:, 128 * qs:128 * qs + 65],
                                aex[:, 512 * h2 + 128 * qs:
                                    512 * h2 + 128 * qs + 128],
                                v1[:, akt, hh, :],
                                start=st and qs == 0, stop=sp,
                                skip_group_check=True)
                    if sp:
                        # pair finished: queue its tail work
                        avA, avB = avt.pop(api)
                        aq0 = 512 * aqb
                        otn = []
                        for qs in range(4):
                            t = otnp.tile([128, 128], bf16, tag="otn",
                                          name=f"otn{api}_{qs}")
                            otn.append(t)
                        recA = recp.tile([128, 4], f32, tag="rec",
                                         name=f"recA{api}")
                        recB = recp.tile([128, 4], f32, tag="rec",
                                         name=f"recB{api}")
                        last = api == NP - 1
                        deferred.append((gi,
                            lambda av=avA, r=recA, o=otn, la=last:
                                emit_norm(av, r, o, 0, on_act=la)))
                        deferred.append((gi + 1,
                            lambda av=avB, r=recB, o=otn:
                                emit_norm(av, r, o, 1, on_act=False)))
                        tpbox = []
                        deferred.append((gi + 2,
                            lambda o=otn, p=ap, q0=aq0, qb=aqb, b=tpbox:
                                b.append(emit_transpose(o, p, q0, qb))))
                        deferred.append((gi + 6,
                            lambda p=ap, q0=aq0, b=tpbox:
                                emit_ot_evac(b[0], p, q0)))
                        if ap == 1:
                            for i4 in range(4):
                                oa = last and i4 % 2 == 0
                                ybox = []
                                deferred.append((gi + 4 + 4 * i4,
                                    lambda stt=4 * aqb + i4, b=ybox:
                                        b.append(emit_fc_mm(stt))))
                                deferred.append((gi + 6 + 4 * i4,
                                    lambda stt=4 * aqb + i4, oa=oa, b=ybox:
                                        emit_fc_out(b[0], stt, on_act=oa)))
            while deferred:
                deferred.pop(0)[1]()

    nc.compile()
    return nc


def _prep(query, key, value, Wq, bq, Wk, bk, Wv, bv, Wfc, bfc):
    """Host-side sharding / layout prep. Returns (in_maps, bfc_eff)."""
    import ml_dtypes

    query = np.asarray(query, dtype=np.float32)
    key = np.asarray(key, dtype=np.float32)
    value = np.asarray(value, dtype=np.float32)
    Wq = np.asarray(Wq, np.float32); bq = np.asarray(bq, np.float32)
    Wk = np.asarray(Wk, np.float32); bk = np.asarray(bk, np.float32)
    Wv = np.asarray(Wv, np.float32); bv = np.asarray(bv, np.float32)
    Wfc = np.asarray(Wfc, np.float32); bfc = np.asarray(bfc, np.float32)

    s_hd = np.float32(1.0 / np.sqrt(HD))
    # fold Wq into the K side: score*log2e = q . (M k) + w . k   (per head)
    M = (Wq.T @ Wk) * (s_hd * LOG2E * ASCALE)          # [d, e]
    w_row = (bq @ Wk) * (s_hd * LOG2E * ASCALE)        # [e]

    # fold Wv / bv into fc
    A = np.empty((D, D), np.float32)
    bfc_eff = bfc.copy()
    for h in range(HEAD):
        Wfc_h = Wfc[:, HD * h:HD * h + HD]
        A[:, HD * h:HD * h + HD] = Wfc_h @ Wv
        bfc_eff += Wfc_h @ bv
    At = np.ascontiguousarray(A.T)                     # [ch, c]

    iden = np.eye(128, dtype=ml_dtypes.bfloat16)

    in_maps = []
    for core in range(N_CORES):
        b, hg = core // 4, core % 4
        ch0 = CH * hg
        qtil = np.zeros((33, 2 * HPC, S), np.float32)
        ktil = np.zeros((33, 2 * HPC, S), np.float32)
        v1 = np.empty((128, NKT, HPC, 65), np.float32)
        for h in range(HPC):
            qh = query[b][:, ch0 + HD * h:ch0 + HD * h + HD]   # [S, 64]
            kh = key[b][:, ch0 + HD * h:ch0 + HD * h + HD]
            kt = kh @ M.T                                      # [S, 64]
            qtil[0:32, 2 * h, :] = qh[:, 0:32].T
            qtil[0:32, 2 * h + 1, :] = qh[:, 32:64].T
            qtil[32, 2 * h, :] = 1.0
            ktil[0:32, 2 * h, :] = kt[:, 0:32].T
            ktil[0:32, 2 * h + 1, :] = kt[:, 32:64].T
            ktil[32, 2 * h, :] = kh @ w_row
            vh = value[b][:, ch0 + HD * h:ch0 + HD * h + HD]
            v1[:, :, h, 0:64] = vh.reshape(NKT, 128, HD).transpose(1, 0, 2)
            v1[:, :, h, 64] = 1.0
        wfc = np.empty((128, 2, D), np.float32)
        wfc[:, 0, :] = At[ch0:ch0 + 128]
        wfc[:, 1, :] = At[ch0 + 128:ch0 + 256]
        qk0 = np.concatenate([ktil[:, 0:4, 0:512], qtil[:, 0:4, 0:512]],
                             axis=1)
        in_maps.append({
            "qk0": qk0.astype(ml_dtypes.float8_e4m3).view(np.uint8),
            "qtil": qtil.astype(ml_dtypes.float8_e4m3).view(np.uint8),
            "ktil": ktil.astype(ml_dtypes.float8_e4m3).view(np.uint8),
            "v1": v1.astype(ml_dtypes.bfloat16).view(np.uint16),
            "wf": wfc.astype(ml_dtypes.bfloat16).view(np.uint16),
            "iden": iden.view(np.uint16),
        })
    return in_maps, bfc_eff


def _run_once(inputs):
    global LAST_RESULTS
    from concourse.bass_utils import run_bass_kernel_spmd

    if "nc" not in _CACHE:
        _CACHE["nc"] = _build()
    nc = _CACHE["nc"]

    in_maps, bfc_eff = _prep(**inputs)
    res = run_bass_kernel_spmd(nc, in_maps, core_ids=list(range(N_CORES)))
    LAST_RESULTS = res

    out = np.empty((B, S, D), np.float32)
    for b in range(B):
        acc = res.results[4 * b]["y"].astype(np.float32).copy()
        for hg in range(1, 4):
            acc += res.results[4 * b + hg]["y"]
        out[b] = acc + bfc_eff
    return out


def kernel(**inputs) -> np.ndarray:
    last_exc = None
    for attempt in range(3):
        try:
            out = _run_once(inputs)
            amax = float(np.abs(out).max())
            if np.isfinite(out).all() and 1e-6 < amax < 1e3:
                return out
            raise RuntimeError(f"implausible kernel output (absmax={amax})")
        except Exception as e:  # noqa: BLE001 - retry transient HW failures
            last_exc = e
            _CACHE.pop("nc", None)
            _CACHE["nonce"] = attempt + 1
    raise last_exc


# revision 45
# speedup vs baseline: 1.0104x; 1.0045x over previous
"""Multi-head attention (B=2, S=2048, D=1024, H=16) on 8 Trainium2 NeuronCores.

Sharding: core i handles batch b = i//4 and head-group hg = i%4 (4 heads).
The fc layer is sharded over its contraction dim (each core emits a partial
y summed on the host); Wv/bv are folded into Wfc/bfc on the host (exact).

Algorithm (v3):
  - Wq is folded into the K side on the host: score = q . k~ with
    k~ = (Wq^T Wk / sqrt(hd)) k + bias-row, so no on-device projections.
    log2(e) and a power-of-two fp8 range scale are folded in as well, so
    the device computes t = score*log2e and exponentiates as 2^t.
  - scores: fp8e4m3 DoubleRow matmuls (contraction 2x33 packs the 64 head
    channels + bias row), out [128 keys, 512 q] psum f32 at 0.5 cycles/row.
  - exp: split per k-tile across the only two PSUM-capable elementwise
    engines (GPSIMD cannot access PSUM on TRN2): ScalarE exact exp->bf16
    (scale=ln2/ascale) for 10/16 tiles, DVE Schraudolph for 6/16 (one
    tensor_scalar mult+add writing int16 exponent-bits through a
    bf16-tile bitcast, ~3% rel err; end-to-end rel err ~1.2e-2).
  - AV transposed: exp tile is the stationary operand [128 keys, 128 q],
    V (bf16, with a ones column for the denominator) streams as moving
    [128, 65] -> out [q, 64ch|den] psum, 65 cycles per k-tile: softmax
    normalization becomes a per-partition reciprocal+scalar-multiply.
  - oTn [128 q, 64A|64B] bf16 pairs are PE-transposed (identity moving)
    to [128 ch, 128 q] and the fc runs K=256 over two bf16 matmuls per
    512-wide psum bank; y is evacuated to SBUF (DVE) and DMA'd out.

Schedule: one flat software-pipelined stream over the 8 (q-window, pair)
units; scores/exp run 3 k-tile steps ahead of the AV consumers. All psum
transients (score tiles, transpose psum, fc psum) share one 3-slot
[128,1024] rotation (6 banks) + 2 banks of AV accumulators; a start=True
matmul zeroes its whole 2KB bank, so interleaved accumulation groups in
one bank carry exactly one start. Tail work (normalize, transpose, oT
evac, fc) is deferred into later stream steps via a not-before-gi queue
so it never blocks the in-order ScalarE/DVE queues at a pair boundary.
"""


import sys

import numpy as np

if "/opt/trn_rl_repo" not in sys.path:
    sys.path.insert(0, "/opt/trn_rl_repo")

HEAD = 16
B, S, D = 2, 2048, 1024
HD = 64
HPC = 4          # heads per core
CH = HPC * HD    # channels per core
N_CORES = 8
NKT = S // 128   # k tiles
NQB = S // 512   # q windows
LOG2E = 1.4426950408889634
ASCALE = 2.0     # fp8 range scale folded into k~; undone in the exp scale

_CACHE = {}
LAST_RESULTS = None


# Per-pair k-tile positions whose exp runs on DVE (Schraudolph); the rest
# run exact exp on ScalarE. 6/16 on DVE globally balances ScalarE's 1038ns
# exp + oT-evac load against DVE's 1193ns exp + y-evac/norm/recip load,
# and keeping kt 0/1 on ScalarE lets DVE run the previous pair's deferred
# normalization immediately at each pair boundary.
# GPSIMD can't touch PSUM on TRN2, so only these two engines qualify.
_DVE_KTS = {1, 4, 6, 9, 11, 14}


def _build():
    import concourse.tile as tile
    from concourse import bacc, mybir

    f32 = mybir.dt.float32
    bf16 = mybir.dt.bfloat16
    fp8 = mybir.dt.float8e4
    i16 = mybir.dt.int16
    EXP = mybir.ActivationFunctionType.Exp
    DR = mybir.MatmulPerfMode.DoubleRow
    MULT = mybir.AluOpType.mult
    ADD = mybir.AluOpType.add

    nc = bacc.Bacc("TRN2", target_bir_lowering=False, debug=False,
                   num_devices=N_CORES)

    # unused internal tensor whose name varies per retry: changes the BIR
    # content hash so a retry never reuses a possibly-corrupt cached NEFF
    nonce = _CACHE.get("nonce", 0)
    if nonce:
        nc.dram_tensor(f"retry_nonce_{nonce}", [1, 1], mybir.dt.float32)

    qt_d = nc.dram_tensor("qtil", [33, 2 * HPC, S], fp8, kind="ExternalInput")
    kt_d = nc.dram_tensor("ktil", [33, 2 * HPC, S], fp8, kind="ExternalInput")
    v1_d = nc.dram_tensor("v1", [128, NKT, HPC, 65], bf16,
                          kind="ExternalInput")
    wf_d = nc.dram_tensor("wf", [128, 2, D], bf16, kind="ExternalInput")
    id_d = nc.dram_tensor("iden", [128, 128], bf16, kind="ExternalInput")
    qk0_d = nc.dram_tensor("qk0", [33, 8, 512], fp8, kind="ExternalInput")
    y_d = nc.dram_tensor("y", [S, D], f32, kind="ExternalOutput")

    act_scale = float(np.log(2.0) / ASCALE)
    sch_mul = float(128.0 / ASCALE)
    sch_add = float(127 * 128 - 5.5)

    with tile.TileContext(nc) as tc, nc.allow_low_precision(
            reason="bf16/fp8 operands feed f32-psum matmuls"):
        with (
            tc.tile_pool(name="inp", bufs=1) as inp,
            tc.tile_pool(name="expp", bufs=8) as expp,
            tc.tile_pool(name="otnp", bufs=4) as otnp,
            tc.tile_pool(name="otp", bufs=1) as otp,
            tc.tile_pool(name="recp", bufs=4) as recp,
            tc.tile_pool(name="ysbp", bufs=4) as ysbp,
            tc.tile_pool(name="ps_big", bufs=3, space="PSUM") as ps_big,
            tc.tile_pool(name="ps_av", bufs=2, space="PSUM") as ps_av,
        ):
            # ------------- inputs (chunked; critical path first) ----------
            qtil = inp.tile([33, 2 * HPC, S], fp8, tag="qtil")
            ktil = inp.tile([33, 2 * HPC, S], fp8, tag="ktil")
            v1 = inp.tile([128, NKT, HPC, 65], bf16, tag="v1")
            wf = inp.tile([128, 2, D], bf16, tag="wf")
            iden = inp.tile([128, 128], bf16, tag="iden")

            # one combined first-chunk DMA (k+q of head-pair 0, first 512
            # keys/queries) so a single transfer gates the first matmul
            qk0 = inp.tile([33, 8, 512], fp8, tag="qk0")
            nc.sync.dma_start(out=qk0, in_=qk0_d[:, :, :])
            nc.sync.dma_start(out=v1[:, 0:4, :, :], in_=v1_d[:, 0:4, :, :])
            nc.sync.dma_start(out=ktil[:, 0:4, 0:512], in_=kt_d[:, 0:4, 0:512])
            nc.sync.dma_start(out=ktil[:, 0:4, 512:2048],
                              in_=kt_d[:, 0:4, 512:2048])
            nc.sync.dma_start(out=qtil[:, 0:4, 0:512], in_=qt_d[:, 0:4, 0:512])
            nc.sync.dma_start(out=iden, in_=id_d[:, :])
            for c in range(1, 8):
                nc.sync.dma_start(out=v1[:, 2 * c:2 * c + 2, :, :],
                                  in_=v1_d[:, 2 * c:2 * c + 2, :, :])
            nc.sync.dma_start(out=ktil[:, 4:8, :], in_=kt_d[:, 4:8, :])
            nc.sync.dma_start(out=qtil[:, 4:8, 0:512], in_=qt_d[:, 4:8, 0:512])
            nc.sync.dma_start(out=qtil[:, :, 512:2048],
                              in_=qt_d[:, :, 512:2048])
            nc.sync.dma_start(out=wf, in_=wf_d[:, :, :])

            oT = []
            for p in range(2):
                t = otp.tile([128, S], bf16, tag=f"oT{p}", name=f"oT{p}")
                oT.append(t)

            # tail work (normalize/transpose/evac/fc) is deferred into the
            # NEXT pair's k-tile stream via a pop queue so it never blocks
            # the in-order Act/DVE queues at a pair boundary
            deferred = []

            def emit_norm(av, rec, otn, h2, on_act=False):
                nc.vector.reciprocal(out=rec, in_=av[:, 64:512:128])
                if on_act:
                    for qs in range(4):
                        nc.scalar.mul(otn[:, 128 * qs + 64 * h2:
                                          128 * qs + 64 * h2 + 64],
                                      av[:, 128 * qs:128 * qs + 64],
                                      rec[:, qs:qs + 1])
                else:
                    # all 4 qs blocks in one op: (av x 1.0) * recip
                    # broadcast along a zero-stride free dim
                    blocks = av[:, :].rearrange("p (a b) -> p a b",
                                                b=128)[:, :, 0:64]
                    ob = otn[:, :].rearrange("p (a b) -> p a b",
                                             b=128)[:, :, 64 * h2:
                                                    64 * h2 + 64]
                    nc.vector.scalar_tensor_tensor(
                        out=ob, in0=blocks, scalar=1.0,
                        in1=rec[:, :].broadcast_to([128, 4, 64]),
                        op0=MULT, op1=MULT)

            def emit_transpose(otn, p, q0, qb):
                tp = ps_big.tile([128, 1024], f32, tag="sc",
                                 name=f"tp{qb}_{p}")[:, 0:256].bitcast(bf16)
                for qs in range(4):
                    nc.tensor.matmul(tp[:, 128 * qs:128 * qs + 128],
                                     otn[:, 128 * qs:128 * qs + 128],
                                     iden[:, :], is_transpose=True,
                                     start=qs == 0, stop=True,
                                     skip_group_check=True)
                return tp

            def emit_ot_evac(tp, p, q0):
                nc.scalar.copy(oT[p][:, q0:q0 + 512], tp)

            def emit_fc_mm(stt):
                # two cb halves: a single matmul may not write across a
                # 2KB psum bank boundary, so the [128,1024] slot is filled
                # by two [128,512] accumulation groups
                yp = ps_big.tile([128, 1024], f32, tag="sc",
                                 name=f"y{stt}")
                for cb in range(2):
                    cs = slice(512 * cb, 512 * cb + 512)
                    nc.tensor.matmul(yp[:, cs],
                                     oT[0][:, 128 * stt:128 * stt + 128],
                                     wf[:, 0, cs], start=True, stop=False,
                                     skip_group_check=True)
                    nc.tensor.matmul(yp[:, cs],
                                     oT[1][:, 128 * stt:128 * stt + 128],
                                     wf[:, 1, cs], start=False, stop=True,
                                     skip_group_check=True)
                return yp

            def emit_fc_out(yp, stt, on_act=False):
                ysb = ysbp.tile([128, 1024], f32, tag="ysb",
                                name=f"ysb{stt}")
                if on_act:
                    nc.scalar.copy(ysb, yp)
                else:
                    nc.vector.tensor_copy(ysb, yp)
                nc.sync.dma_start(
                    out=y_d[128 * stt:128 * stt + 128, :], in_=ysb)

            # prewarm the Exp activation table during the input DMAs
            warm = recp.tile([128, 1], f32, tag="warm")
            nc.vector.memset(warm, 0.0)
            nc.scalar.activation(out=warm, in_=warm, func=EXP,
                                 scale=act_scale)

            # single flat software-pipelined stream over all 8 (qb, p)
            # pairs: scores/exp run 2 k-tile steps ahead of the AV
            # consumers, crossing pair boundaries without a bubble
            pairs = [(qb, p) for qb in range(NQB) for p in range(2)]
            NP = len(pairs)
            pend = []
            avt = {}
            for gi in range(NP * NKT + 3):
                while deferred and deferred[0][0] <= gi:
                    deferred.pop(0)[1]()
                if gi < NP * NKT:
                    pi, kt = divmod(gi, NKT)
                    qb, p = pairs[pi]
                    q0 = 512 * qb
                    hA, hB = 2 * p, 2 * p + 1
                    ks = slice(128 * kt, 128 * kt + 128)
                    sc = ps_big.tile([128, 1024], f32, tag="sc",
                                    name=f"sc{qb}_{p}_{kt}")
                    if pi == 0 and kt < 4:
                        kA, qA = qk0[:, 0:2, ks], qk0[:, 4:6, :]
                        kB, qB = qk0[:, 2:4, ks], qk0[:, 6:8, :]
                    else:
                        kA = ktil[:, 2 * hA:2 * hA + 2, ks]
                        qA = qtil[:, 2 * hA:2 * hA + 2, q0:q0 + 512]
                        kB = ktil[:, 2 * hB:2 * hB + 2, ks]
                        qB = qtil[:, 2 * hB:2 * hB + 2, q0:q0 + 512]
                    nc.tensor.matmul(sc[:, 0:512], kA, qA,
                                     start=True, stop=True, perf_mode=DR)
                    nc.tensor.matmul(sc[:, 512:1024], kB, qB,
                                     start=True, stop=True, perf_mode=DR)
                    ex = expp.tile([128, 1024], bf16, tag="ex",
                                   name=f"ex{qb}_{p}_{kt}")
                    if kt not in _DVE_KTS:
                        nc.scalar.activation(out=ex, in_=sc, func=EXP,
                                             scale=act_scale)
                    else:
                        nc.vector.tensor_scalar(
                            out=ex.bitcast(i16), in0=sc,
                            scalar1=sch_mul, scalar2=sch_add,
                            op0=MULT, op1=ADD)
                    pend.append((pi, kt, ex))
                if gi >= 3:
                    api, akt, aex = pend[gi - 3]
                    aqb, ap = pairs[api]
                    if akt == 0:
                        # one 2KB bank per accumulator; qs blocks padded to
                        # 128 floats so only the qs==0 matmul carries
                        # start=True (a start marks the whole 2KB
                        # zero-region, so interleaved groups must share a
                        # single start per bank)
                        avt[api] = (
                            ps_av.tile([128, 512], f32, tag="av",
                                       name=f"avA{api}"),
                            ps_av.tile([128, 512], f32, tag="av",
                                       name=f"avB{api}"))
                    st = akt == 0
                    sp = akt == NKT - 1
                    for h2, hh in ((0, 2 * ap), (1, 2 * ap + 1)):
                        av = avt[api][h2]
                        for qs in range(4):
                            nc.tensor.matmul(
                                av[:, 128 * qs:128 * qs + 65],
                                aex[:, 512 * h2 + 128 * qs:
                                    512 * h2 + 128 * qs + 128],
                                v1[:, akt, hh, :],
                                start=st and qs == 0, stop=sp,
                                skip_group_check=True)
                    if sp:
                        # pair finished: queue its tail work
                        avA, avB = avt.pop(api)
                        aq0 = 512 * aqb
                        otn = otnp.tile([128, 512], bf16, tag="otn",
                                        name=f"otn{api}")
                        recA = recp.tile([128, 4], f32, tag="rec",
                                         name=f"recA{api}")
                        recB = recp.tile([128, 4], f32, tag="rec",
                                         name=f"recB{api}")
                        last = api == NP - 1
                        deferred.append((gi,
                            lambda av=avA, r=recA, o=otn, la=last:
                                emit_norm(av, r, o, 0, on_act=la)))
                        deferred.append((gi + 1,
                            lambda av=avB, r=recB, o=otn:
                                emit_norm(av, r, o, 1, on_act=False)))
                        tpbox = []
                        deferred.append((gi + 2,
                            lambda o=otn, p=ap, q0=aq0, qb=aqb, b=tpbox:
                                b.append(emit_transpose(o, p, q0, qb))))
                        deferred.append((gi + 6,
                            lambda p=ap, q0=aq0, b=tpbox:
                                emit_ot_evac(b[0], p, q0)))
                        if ap == 1:
                            for i4 in range(4):
                                oa = last and i4 % 2 == 0
                                ybox = []
                                deferred.append((gi + 4 + 4 * i4,
                                    lambda stt=4 * aqb + i4, b=ybox:
                                        b.append(emit_fc_mm(stt))))
                                deferred.append((gi + 6 + 4 * i4,
                                    lambda stt=4 * aqb + i4, oa=oa, b=ybox:
                                        emit_fc_out(b[0], stt, on_act=oa)))
            while deferred:
                deferred.pop(0)[1]()

    nc.compile()
    return nc


def _prep(query, key, value, Wq, bq, Wk, bk, Wv, bv, Wfc, bfc):
    """Host-side sharding / layout prep. Returns (in_maps, bfc_eff)."""
    import ml_dtypes

    query = np.asarray(query, dtype=np.float32)
    key = np.asarray(key, dtype=np.float32)
    value = np.asarray(value, dtype=np.float32)
    Wq = np.asarray(Wq, np.float32); bq = np.asarray(bq, np.float32)
    Wk = np.asarray(Wk, np.float32); bk = np.asarray(bk, np.float32)
    Wv = np.asarray(Wv, np.float32); bv = np.asarray(bv, np.float32)
    Wfc = np.asarray(Wfc, np.float32); bfc = np.asarray(bfc, np.float32)

    s_hd = np.float32(1.0 / np.sqrt(HD))
    # fold Wq into the K side: score*log2e = q . (M k) + w . k   (per head)
    M = (Wq.T @ Wk) * (s_hd * LOG2E * ASCALE)          # [d, e]
    w_row = (bq @ Wk) * (s_hd * LOG2E * ASCALE)        # [e]

    # fold Wv / bv into fc
    A = np.empty((D, D), np.float32)
    bfc_eff = bfc.copy()
    for h in range(HEAD):
        Wfc_h = Wfc[:, HD * h:HD * h + HD]
        A[:, HD * h:HD * h + HD] = Wfc_h @ Wv
        bfc_eff += Wfc_h @ bv
    At = np.ascontiguousarray(A.T)                     # [ch, c]

    iden = np.eye(128, dtype=ml_dtypes.bfloat16)

    in_maps = []
    for core in range(N_CORES):
        b, hg = core // 4, core % 4
        ch0 = CH * hg
        qtil = np.zeros((33, 2 * HPC, S), np.float32)
        ktil = np.zeros((33, 2 * HPC, S), np.float32)
        v1 = np.empty((128, NKT, HPC, 65), np.float32)
        for h in range(HPC):
            qh = query[b][:, ch0 + HD * h:ch0 + HD * h + HD]   # [S, 64]
            kh = key[b][:, ch0 + HD * h:ch0 + HD * h + HD]
            kt = kh @ M.T                                      # [S, 64]
            qtil[0:32, 2 * h, :] = qh[:, 0:32].T
            qtil[0:32, 2 * h + 1, :] = qh[:, 32:64].T
            qtil[32, 2 * h, :] = 1.0
            ktil[0:32, 2 * h, :] = kt[:, 0:32].T
            ktil[0:32, 2 * h + 1, :] = kt[:, 32:64].T
            ktil[32, 2 * h, :] = kh @ w_row
            vh = value[b][:, ch0 + HD * h:ch0 + HD * h + HD]
            v1[:, :, h, 0:64] = vh.reshape(NKT, 128, HD).transpose(1, 0, 2)
            v1[:, :, h, 64] = 1.0
        wfc = np.empty((128, 2, D), np.float32)
        wfc[:, 0, :] = At[ch0:ch0 + 128]
        wfc[:, 1, :] = At[ch0 + 128:ch0 + 256]
        qk0 = np.concatenate([ktil[:, 0:4, 0:512], qtil[:, 0:4, 0:512]],
                             axis=1)
        in_maps.append({
            "qk0": qk0.astype(ml_dtypes.float8_e4m3).view(np.uint8),
            "qtil": qtil.astype(ml_dtypes.float8_e4m3).view(np.uint8),
            "ktil": ktil.astype(ml_dtypes.float8_e4m3).view(np.uint8),
            "v1": v1.astype(ml_dtypes.bfloat16).view(np.uint16),
            "wf": wfc.astype(ml_dtypes.bfloat16).view(np.uint16),
            "iden": iden.view(np.uint16),
        })
    return in_maps, bfc_eff


def _run_once(inputs):
    global LAST_RESULTS
    from concourse.bass_utils import run_bass_kernel_spmd

    if "nc" not in _CACHE:
        _CACHE["nc"] = _build()
    nc = _CACHE["nc"]

    in_maps, bfc_eff = _prep(**inputs)
    res = run_bass_kernel_spmd(nc, in_maps, core_ids=list(range(N_CORES)))
    LAST_RESULTS = res

    out = np.empty((B, S, D), np.float32)
    for b in range(B):
        acc = res.results[4 * b]["y"].astype(np.float32).copy()
        for hg in range(1, 4):
            acc += res.results[4 * b + hg]["y"]
        out[b] = acc + bfc_eff
    return out


def kernel(**inputs) -> np.ndarray:
    last_exc = None
    for attempt in range(3):
        try:
            out = _run_once(inputs)
            amax = float(np.abs(out).max())
            if np.isfinite(out).all() and 1e-6 < amax < 1e3:
                return out
            raise RuntimeError(f"implausible kernel output (absmax={amax})")
        except Exception as e:  # noqa: BLE001 - retry transient HW failures
            last_exc = e
            _CACHE.pop("nc", None)
            _CACHE["nonce"] = attempt + 1
    raise last_exc


# revision 47
# speedup vs baseline: 1.0272x; 1.0166x over previous
"""Multi-head attention (B=2, S=2048, D=1024, H=16) on 8 Trainium2 NeuronCores.

Sharding: core i handles batch b = i//4 and head-group hg = i%4 (4 heads).
The fc layer is sharded over its contraction dim (each core emits a partial
y summed on the host); Wv/bv are folded into Wfc/bfc on the host (exact).

Algorithm (v3):
  - Wq is folded into the K side on the host: score = q . k~ with
    k~ = (Wq^T Wk / sqrt(hd)) k + bias-row, so no on-device projections.
    log2(e) and a power-of-two fp8 range scale are folded in as well, so
    the device computes t = score*log2e and exponentiates as 2^t.
  - scores: fp8e4m3 DoubleRow matmuls (contraction 2x33 packs the 64 head
    channels + bias row), out [128 keys, 512 q] psum f32 at 0.5 cycles/row.
  - exp: split per k-tile across the only two PSUM-capable elementwise
    engines (GPSIMD cannot access PSUM on TRN2): ScalarE exact exp->bf16
    (scale=ln2/ascale) for 10/16 tiles, DVE Schraudolph for 6/16 (one
    tensor_scalar mult+add writing int16 exponent-bits through a
    bf16-tile bitcast, ~3% rel err; end-to-end rel err ~1.2e-2).
  - AV transposed: exp tile is the stationary operand [128 keys, 128 q],
    V (bf16, with a ones column for the denominator) streams as moving
    [128, 65] -> out [q, 64ch|den] psum, 65 cycles per k-tile: softmax
    normalization becomes a per-partition reciprocal+scalar-multiply.
  - oTn [128 q, 64A|64B] bf16 pairs are PE-transposed (identity moving)
    to [128 ch, 128 q] and the fc runs K=256 over two bf16 matmuls per
    512-wide psum bank; y is evacuated to SBUF (DVE) and DMA'd out.

Schedule: one flat software-pipelined stream over the 8 (q-window, pair)
units; scores/exp run 3 k-tile steps ahead of the AV consumers. All psum
transients (score tiles, transpose psum, fc psum) share one 3-slot
[128,1024] rotation (6 banks) + 2 banks of AV accumulators; a start=True
matmul zeroes its whole 2KB bank, so interleaved accumulation groups in
one bank carry exactly one start. Tail work (normalize, transpose, oT
evac, fc) is deferred into later stream steps via a not-before-gi queue
so it never blocks the in-order ScalarE/DVE queues at a pair boundary.
"""


import sys

import numpy as np

if "/opt/trn_rl_repo" not in sys.path:
    sys.path.insert(0, "/opt/trn_rl_repo")

HEAD = 16
B, S, D = 2, 2048, 1024
HD = 64
HPC = 4          # heads per core
CH = HPC * HD    # channels per core
N_CORES = 8
NKT = S // 128   # k tiles
NQB = S // 512   # q windows
LOG2E = 1.4426950408889634
ASCALE = 2.0     # fp8 range scale folded into k~; undone in the exp scale

_CACHE = {}
LAST_RESULTS = None


# Per-pair k-tile positions whose exp runs on DVE (Schraudolph); the rest
# run exact exp on ScalarE. 6/16 on DVE globally balances ScalarE's 1038ns
# exp + oT-evac load against DVE's 1193ns exp + y-evac/norm/recip load,
# and keeping kt 0/1 on ScalarE lets DVE run the previous pair's deferred
# normalization immediately at each pair boundary.
# GPSIMD can't touch PSUM on TRN2, so only these two engines qualify.
_DVE_KTS = {1, 4, 6, 9, 11, 14}


def _build():
    import concourse.tile as tile
    from concourse import bacc, mybir

    f32 = mybir.dt.float32
    bf16 = mybir.dt.bfloat16
    fp8 = mybir.dt.float8e4
    i16 = mybir.dt.int16
    EXP = mybir.ActivationFunctionType.Exp
    DR = mybir.MatmulPerfMode.DoubleRow
    MULT = mybir.AluOpType.mult
    ADD = mybir.AluOpType.add

    nc = bacc.Bacc("TRN2", target_bir_lowering=False, debug=False,
                   num_devices=N_CORES)

    # unused internal tensor whose name varies per retry: changes the BIR
    # content hash so a retry never reuses a possibly-corrupt cached NEFF
    nonce = _CACHE.get("nonce", 0)
    if nonce:
        nc.dram_tensor(f"retry_nonce_{nonce}", [1, 1], mybir.dt.float32)

    qt_d = nc.dram_tensor("qtil", [33, 2 * HPC, S], fp8, kind="ExternalInput")
    kt_d = nc.dram_tensor("ktil", [33, 2 * HPC, S], fp8, kind="ExternalInput")
    v1_d = nc.dram_tensor("v1", [128, NKT, HPC, 65], bf16,
                          kind="ExternalInput")
    wf_d = nc.dram_tensor("wf", [128, 2, D], bf16, kind="ExternalInput")
    id_d = nc.dram_tensor("iden", [128, 128], bf16, kind="ExternalInput")
    qk0_d = nc.dram_tensor("qk0", [33, 8, 512], fp8, kind="ExternalInput")
    y_d = nc.dram_tensor("y", [S, D], f32, kind="ExternalOutput")

    act_scale = float(np.log(2.0) / ASCALE)
    sch_mul = float(128.0 / ASCALE)
    sch_add = float(127 * 128 - 5.5)

    with tile.TileContext(nc) as tc, nc.allow_low_precision(
            reason="bf16/fp8 operands feed f32-psum matmuls"):
        with (
            tc.tile_pool(name="inp", bufs=1) as inp,
            tc.tile_pool(name="expp", bufs=8) as expp,
            tc.tile_pool(name="otnp", bufs=4) as otnp,
            tc.tile_pool(name="otp", bufs=1) as otp,
            tc.tile_pool(name="recp", bufs=4) as recp,
            tc.tile_pool(name="ysbp", bufs=4) as ysbp,
            tc.tile_pool(name="ps_big", bufs=3, space="PSUM") as ps_big,
            tc.tile_pool(name="ps_av", bufs=2, space="PSUM") as ps_av,
        ):
            # ------------- inputs (chunked; critical path first) ----------
            qtil = inp.tile([33, 2 * HPC, S], fp8, tag="qtil")
            ktil = inp.tile([33, 2 * HPC, S], fp8, tag="ktil")
            v1 = inp.tile([128, NKT, HPC, 65], bf16, tag="v1")
            wf = inp.tile([128, 2, D], bf16, tag="wf")
            iden = inp.tile([128, 128], bf16, tag="iden")

            # one combined first-chunk DMA (k+q of head-pair 0, first 512
            # keys/queries) so a single transfer gates the first matmul
            qk0 = inp.tile([33, 8, 512], fp8, tag="qk0")
            nc.sync.dma_start(out=qk0, in_=qk0_d[:, :, :])
            nc.sync.dma_start(out=v1[:, 0:4, :, :], in_=v1_d[:, 0:4, :, :])
            nc.sync.dma_start(out=ktil[:, 0:4, 0:512], in_=kt_d[:, 0:4, 0:512])
            nc.sync.dma_start(out=ktil[:, 0:4, 512:2048],
                              in_=kt_d[:, 0:4, 512:2048])
            nc.sync.dma_start(out=qtil[:, 0:4, 0:512], in_=qt_d[:, 0:4, 0:512])
            nc.sync.dma_start(out=iden, in_=id_d[:, :])
            for c in range(1, 8):
                nc.sync.dma_start(out=v1[:, 2 * c:2 * c + 2, :, :],
                                  in_=v1_d[:, 2 * c:2 * c + 2, :, :])
            nc.sync.dma_start(out=ktil[:, 4:8, :], in_=kt_d[:, 4:8, :])
            nc.sync.dma_start(out=qtil[:, 4:8, 0:512], in_=qt_d[:, 4:8, 0:512])
            nc.sync.dma_start(out=qtil[:, :, 512:2048],
                              in_=qt_d[:, :, 512:2048])
            nc.sync.dma_start(out=wf, in_=wf_d[:, :, :])

            oT = []
            for p in range(2):
                t = otp.tile([128, S], bf16, tag=f"oT{p}", name=f"oT{p}")
                oT.append(t)

            # tail work (normalize/transpose/evac/fc) is deferred into the
            # NEXT pair's k-tile stream via a pop queue so it never blocks
            # the in-order Act/DVE queues at a pair boundary
            deferred = []

            def emit_norm(av, rec, otn, h2, on_act=False):
                nc.vector.reciprocal(out=rec, in_=av[:, 64:512:128])
                if on_act:
                    for qs in range(4):
                        nc.scalar.mul(otn[:, 128 * qs + 64 * h2:
                                          128 * qs + 64 * h2 + 64],
                                      av[:, 128 * qs:128 * qs + 64],
                                      rec[:, qs:qs + 1])
                else:
                    # all 4 qs blocks in one op: (av x 1.0) * recip
                    # broadcast along a zero-stride free dim
                    blocks = av[:, :].rearrange("p (a b) -> p a b",
                                                b=128)[:, :, 0:64]
                    ob = otn[:, :].rearrange("p (a b) -> p a b",
                                             b=128)[:, :, 64 * h2:
                                                    64 * h2 + 64]
                    nc.vector.scalar_tensor_tensor(
                        out=ob, in0=blocks, scalar=1.0,
                        in1=rec[:, :].broadcast_to([128, 4, 64]),
                        op0=MULT, op1=MULT)

            def emit_transpose(otn, p, q0, qb):
                tp = ps_big.tile([128, 1024], f32, tag="sc",
                                 name=f"tp{qb}_{p}")[:, 0:256].bitcast(bf16)
                for qs in range(4):
                    nc.tensor.matmul(tp[:, 128 * qs:128 * qs + 128],
                                     otn[:, 128 * qs:128 * qs + 128],
                                     iden[:, :], is_transpose=True,
                                     start=qs == 0, stop=True,
                                     skip_group_check=True)
                return tp

            def emit_ot_evac(tp, p, q0):
                nc.scalar.copy(oT[p][:, q0:q0 + 512], tp)

            def emit_fc_mm(stt):
                # two cb halves: a single matmul may not write across a
                # 2KB psum bank boundary, so the [128,1024] slot is filled
                # by two [128,512] accumulation groups
                yp = ps_big.tile([128, 1024], f32, tag="sc",
                                 name=f"y{stt}")
                for cb in range(2):
                    cs = slice(512 * cb, 512 * cb + 512)
                    nc.tensor.matmul(yp[:, cs],
                                     oT[0][:, 128 * stt:128 * stt + 128],
                                     wf[:, 0, cs], start=True, stop=False,
                                     skip_group_check=True)
                    nc.tensor.matmul(yp[:, cs],
                                     oT[1][:, 128 * stt:128 * stt + 128],
                                     wf[:, 1, cs], start=False, stop=True,
                                     skip_group_check=True)
                return yp

            def emit_fc_out(yp, stt, on_act=False):
                ysb = ysbp.tile([128, 1024], f32, tag="ysb",
                                name=f"ysb{stt}")
                if on_act:
                    nc.scalar.copy(ysb, yp)
                else:
                    nc.vector.tensor_copy(ysb, yp)
                nc.sync.dma_start(
                    out=y_d[128 * stt:128 * stt + 128, :], in_=ysb)

            # prewarm the Exp activation table during the input DMAs
            warm = recp.tile([128, 1], f32, tag="warm")
            nc.vector.memset(warm, 0.0)
            nc.scalar.activation(out=warm, in_=warm, func=EXP,
                                 scale=act_scale)

            # single flat software-pipelined stream over all 8 (qb, p)
            # pairs: scores/exp run 2 k-tile steps ahead of the AV
            # consumers, crossing pair boundaries without a bubble
            pairs = [(qb, p) for qb in range(NQB) for p in range(2)]
            NP = len(pairs)
            pend = []
            avt = {}
            for gi in range(NP * NKT + 3):
                while deferred and deferred[0][0] <= gi:
                    deferred.pop(0)[1]()
                if gi < NP * NKT:
                    pi, kt = divmod(gi, NKT)
                    qb, p = pairs[pi]
                    q0 = 512 * qb
                    hA, hB = 2 * p, 2 * p + 1
                    ks = slice(128 * kt, 128 * kt + 128)
                    sc = ps_big.tile([128, 1024], f32, tag="sc",
                                    name=f"sc{qb}_{p}_{kt}")
                    if pi == 0 and kt < 4:
                        kA, qA = qk0[:, 0:2, ks], qk0[:, 4:6, :]
                        kB, qB = qk0[:, 2:4, ks], qk0[:, 6:8, :]
                    else:
                        kA = ktil[:, 2 * hA:2 * hA + 2, ks]
                        qA = qtil[:, 2 * hA:2 * hA + 2, q0:q0 + 512]
                        kB = ktil[:, 2 * hB:2 * hB + 2, ks]
                        qB = qtil[:, 2 * hB:2 * hB + 2, q0:q0 + 512]
                    nc.tensor.matmul(sc[:, 0:512], kA, qA,
                                     start=True, stop=True, perf_mode=DR)
                    nc.tensor.matmul(sc[:, 512:1024], kB, qB,
                                     start=True, stop=True, perf_mode=DR)
                    ex = expp.tile([128, 1024], bf16, tag="ex",
                                   name=f"ex{qb}_{p}_{kt}")
                    if kt not in _DVE_KTS:
                        nc.scalar.activation(out=ex, in_=sc, func=EXP,
                                             scale=act_scale)
                    else:
                        nc.vector.tensor_scalar(
                            out=ex.bitcast(i16), in0=sc,
                            scalar1=sch_mul, scalar2=sch_add,
                            op0=MULT, op1=ADD)
                    pend.append((pi, kt, ex))
                if gi >= 3:
                    api, akt, aex = pend[gi - 3]
                    aqb, ap = pairs[api]
                    if akt == 0:
                        # one 2KB bank per accumulator; qs blocks padded to
                        # 128 floats so only the qs==0 matmul carries
                        # start=True (a start marks the whole 2KB
                        # zero-region, so interleaved groups must share a
                        # single start per bank)
                        avt[api] = (
                            ps_av.tile([128, 512], f32, tag="av",
                                       name=f"avA{api}"),
                            ps_av.tile([128, 512], f32, tag="av",
                                       name=f"avB{api}"))
                    st = akt == 0
                    sp = akt == NKT - 1
                    for h2, hh in ((0, 2 * ap), (1, 2 * ap + 1)):
                        av = avt[api][h2]
                        for qs in range(4):
                            nc.tensor.matmul(
                                av[:, 128 * qs:128 * qs + 65],
                                aex[:, 512 * h2 + 128 * qs:
                                    512 * h2 + 128 * qs + 128],
                                v1[:, akt, hh, :],
                                start=st and qs == 0, stop=sp,
                                skip_group_check=True)
                    if sp:
                        # pair finished: queue its tail work
                        avA, avB = avt.pop(api)
                        aq0 = 512 * aqb
                        otn = otnp.tile([128, 512], bf16, tag="otn",
                                        name=f"otn{api}")
                        recA = recp.tile([128, 4], f32, tag="rec",
                                         name=f"recA{api}")
                        recB = recp.tile([128, 4], f32, tag="rec",
                                         name=f"recB{api}")
                        last = api == NP - 1
                        deferred.append((gi,
                            lambda av=avA, r=recA, o=otn, la=last:
                                emit_norm(av, r, o, 0, on_act=la)))
                        deferred.append((gi + 1,
                            lambda av=avB, r=recB, o=otn:
                                emit_norm(av, r, o, 1, on_act=False)))
                        tpbox = []
                        deferred.append((gi + 2,
                            lambda o=otn, p=ap, q0=aq0, qb=aqb, b=tpbox:
                                b.append(emit_transpose(o, p, q0, qb))))
                        deferred.append((gi + 6,
                            lambda p=ap, q0=aq0, b=tpbox:
                                emit_ot_evac(b[0], p, q0)))
                        if ap == 1:
                            for i4 in range(4):
                                oa = i4 % 3 == 0
                                ybox = []
                                deferred.append((gi + 4 + 4 * i4,
                                    lambda stt=4 * aqb + i4, b=ybox:
                                        b.append(emit_fc_mm(stt))))
                                deferred.append((gi + 6 + 4 * i4,
                                    lambda stt=4 * aqb + i4, oa=oa, b=ybox:
                                        emit_fc_out(b[0], stt, on_act=oa)))
            while deferred:
                deferred.pop(0)[1]()

    nc.compile()
    return nc


def _prep(query, key, value, Wq, bq, Wk, bk, Wv, bv, Wfc, bfc):
    """Host-side sharding / layout prep. Returns (in_maps, bfc_eff)."""
    import ml_dtypes

    query = np.asarray(query, dtype=np.float32)
    key = np.asarray(key, dtype=np.float32)
    value = np.asarray(value, dtype=np.float32)
    Wq = np.asarray(Wq, np.float32); bq = np.asarray(bq, np.float32)
    Wk = np.asarray(Wk, np.float32); bk = np.asarray(bk, np.float32)
    Wv = np.asarray(Wv, np.float32); bv = np.asarray(bv, np.float32)
    Wfc = np.asarray(Wfc, np.float32); bfc = np.asarray(bfc, np.float32)

    s_hd = np.float32(1.0 / np.sqrt(HD))
    # fold Wq into the K side: score*log2e = q . (M k) + w . k   (per head)
    M = (Wq.T @ Wk) * (s_hd * LOG2E * ASCALE)          # [d, e]
    w_row = (bq @ Wk) * (s_hd * LOG2E * ASCALE)        # [e]

    # fold Wv / bv into fc
    A = np.empty((D, D), np.float32)
    bfc_eff = bfc.copy()
    for h in range(HEAD):
        Wfc_h = Wfc[:, HD * h:HD * h + HD]
        A[:, HD * h:HD * h + HD] = Wfc_h @ Wv
        bfc_eff += Wfc_h @ bv
    At = np.ascontiguousarray(A.T)                     # [ch, c]

    iden = np.eye(128, dtype=ml_dtypes.bfloat16)

    in_maps = []
    for core in range(N_CORES):
        b, hg = core // 4, core % 4
        ch0 = CH * hg
        qtil = np.zeros((33, 2 * HPC, S), np.float32)
        ktil = np.zeros((33, 2 * HPC, S), np.float32)
        v1 = np.empty((128, NKT, HPC, 65), np.float32)
        for h in range(HPC):
            qh = query[b][:, ch0 + HD * h:ch0 + HD * h + HD]   # [S, 64]
            kh = key[b][:, ch0 + HD * h:ch0 + HD * h + HD]
            kt = kh @ M.T                                      # [S, 64]
            qtil[0:32, 2 * h, :] = qh[:, 0:32].T
            qtil[0:32, 2 * h + 1, :] = qh[:, 32:64].T
            qtil[32, 2 * h, :] = 1.0
            ktil[0:32, 2 * h, :] = kt[:, 0:32].T
            ktil[0:32, 2 * h + 1, :] = kt[:, 32:64].T
            ktil[32, 2 * h, :] = kh @ w_row
            vh = value[b][:, ch0 + HD * h:ch0 + HD * h + HD]
            v1[:, :, h, 0:64] = vh.reshape(NKT, 128, HD).transpose(1, 0, 2)
            v1[:, :, h, 64] = 1.0
        wfc = np.empty((128, 2, D), np.float32)
        wfc[:, 0, :] = At[ch0:ch0 + 128]
        wfc[:, 1, :] = At[ch0 + 128:ch0 + 256]
        qk0 = np.concatenate([ktil[:, 0:4, 0:512], qtil[:, 0:4, 0:512]],
                             axis=1)
        in_maps.append({
            "qk0": qk0.astype(ml_dtypes.float8_e4m3).view(np.uint8),
            "qtil": qtil.astype(ml_dtypes.float8_e4m3).view(np.uint8),
            "ktil": ktil.astype(ml_dtypes.float8_e4m3).view(np.uint8),
            "v1": v1.astype(ml_dtypes.bfloat16).view(np.uint16),
            "wf": wfc.astype(ml_dtypes.bfloat16).view(np.uint16),
            "iden": iden.view(np.uint16),
        })
    return in_maps, bfc_eff


def _run_once(inputs):
    global LAST_RESULTS
    from concourse.bass_utils import run_bass_kernel_spmd

    if "nc" not in _CACHE:
        _CACHE["nc"] = _build()
    nc = _CACHE["nc"]

    in_maps, bfc_eff = _prep(**inputs)
    res = run_bass_kernel_spmd(nc, in_maps, core_ids=list(range(N_CORES)))
    LAST_RESULTS = res

    out = np.empty((B, S, D), np.float32)
    for b in range(B):
        acc = res.results[4 * b]["y"].astype(np.float32).copy()
        for hg in range(1, 4):
            acc += res.results[4 * b + hg]["y"]
        out[b] = acc + bfc_eff
    return out


def kernel(**inputs) -> np.ndarray:
    last_exc = None
    for attempt in range(3):
        try:
            out = _run_once(inputs)
            amax = float(np.abs(out).max())
            if np.isfinite(out).all() and 1e-6 < amax < 1e3:
                return out
            raise RuntimeError(f"implausible kernel output (absmax={amax})")
        except Exception as e:  # noqa: BLE001 - retry transient HW failures
            last_exc = e
            _CACHE.pop("nc", None)
            _CACHE["nonce"] = attempt + 1
    raise last_exc


# revision 48
# speedup vs baseline: 1.0487x; 1.0209x over previous
"""Multi-head attention (B=2, S=2048, D=1024, H=16) on 8 Trainium2 NeuronCores.

Sharding: core i handles batch b = i//4 and head-group hg = i%4 (4 heads).
The fc layer is sharded over its contraction dim (each core emits a partial
y summed on the host); Wv/bv are folded into Wfc/bfc on the host (exact).

Algorithm (v3):
  - Wq is folded into the K side on the host: score = q . k~ with
    k~ = (Wq^T Wk / sqrt(hd)) k + bias-row, so no on-device projections.
    log2(e) and a power-of-two fp8 range scale are folded in as well, so
    the device computes t = score*log2e and exponentiates as 2^t.
  - scores: fp8e4m3 DoubleRow matmuls (contraction 2x33 packs the 64 head
    channels + bias row), out [128 keys, 512 q] psum f32 at 0.5 cycles/row.
  - exp: split per k-tile across the only two PSUM-capable elementwise
    engines (GPSIMD cannot access PSUM on TRN2): ScalarE exact exp->bf16
    (scale=ln2/ascale) for 10/16 tiles, DVE Schraudolph for 6/16 (one
    tensor_scalar mult+add writing int16 exponent-bits through a
    bf16-tile bitcast, ~3% rel err; end-to-end rel err ~1.2e-2).
  - AV transposed: exp tile is the stationary operand [128 keys, 128 q],
    V (bf16, with a ones column for the denominator) streams as moving
    [128, 65] -> out [q, 64ch|den] psum, 65 cycles per k-tile: softmax
    normalization becomes a per-partition reciprocal+scalar-multiply.
  - oTn [128 q, 64A|64B] bf16 pairs are PE-transposed (identity moving)
    to [128 ch, 128 q] and the fc runs K=256 over two bf16 matmuls per
    512-wide psum bank; y is evacuated to SBUF (DVE) and DMA'd out.

Schedule: one flat software-pipelined stream over the 8 (q-window, pair)
units; scores/exp run 3 k-tile steps ahead of the AV consumers. All psum
transients (score tiles, transpose psum, fc psum) share one 3-slot
[128,1024] rotation (6 banks) + 2 banks of AV accumulators; a start=True
matmul zeroes its whole 2KB bank, so interleaved accumulation groups in
one bank carry exactly one start. Tail work (normalize, transpose, oT
evac, fc) is deferred into later stream steps via a not-before-gi queue
so it never blocks the in-order ScalarE/DVE queues at a pair boundary.
"""


import sys

import numpy as np

if "/opt/trn_rl_repo" not in sys.path:
    sys.path.insert(0, "/opt/trn_rl_repo")

HEAD = 16
B, S, D = 2, 2048, 1024
HD = 64
HPC = 4          # heads per core
CH = HPC * HD    # channels per core
N_CORES = 8
NKT = S // 128   # k tiles
NQB = S // 512   # q windows
LOG2E = 1.4426950408889634
ASCALE = 2.0     # fp8 range scale folded into k~; undone in the exp scale

_CACHE = {}
LAST_RESULTS = None


# Per-pair k-tile positions whose exp runs on DVE (Schraudolph); the rest
# run exact exp on ScalarE. 6/16 on DVE globally balances ScalarE's 1038ns
# exp + oT-evac load against DVE's 1193ns exp + y-evac/norm/recip load,
# and keeping kt 0/1 on ScalarE lets DVE run the previous pair's deferred
# normalization immediately at each pair boundary.
# GPSIMD can't touch PSUM on TRN2, so only these two engines qualify.
_DVE_KTS = {1, 4, 6, 9, 11, 14}


def _build():
    import concourse.tile as tile
    from concourse import bacc, mybir

    f32 = mybir.dt.float32
    bf16 = mybir.dt.bfloat16
    fp8 = mybir.dt.float8e4
    i16 = mybir.dt.int16
    EXP = mybir.ActivationFunctionType.Exp
    DR = mybir.MatmulPerfMode.DoubleRow
    MULT = mybir.AluOpType.mult
    ADD = mybir.AluOpType.add

    nc = bacc.Bacc("TRN2", target_bir_lowering=False, debug=False,
                   num_devices=N_CORES)

    # unused internal tensor whose name varies per retry: changes the BIR
    # content hash so a retry never reuses a possibly-corrupt cached NEFF
    nonce = _CACHE.get("nonce", 0)
    if nonce:
        nc.dram_tensor(f"retry_nonce_{nonce}", [1, 1], mybir.dt.float32)

    qt_d = nc.dram_tensor("qtil", [33, 2 * HPC, S], fp8, kind="ExternalInput")
    kt_d = nc.dram_tensor("ktil", [33, 2 * HPC, S], fp8, kind="ExternalInput")
    v1_d = nc.dram_tensor("v1", [128, NKT, HPC, 65], bf16,
                          kind="ExternalInput")
    wf_d = nc.dram_tensor("wf", [128, 2, D], bf16, kind="ExternalInput")
    id_d = nc.dram_tensor("iden", [128, 128], bf16, kind="ExternalInput")
    qk0_d = nc.dram_tensor("qk0", [33, 8, 512], fp8, kind="ExternalInput")
    y_d = nc.dram_tensor("y", [S, D], bf16, kind="ExternalOutput")

    act_scale = float(np.log(2.0) / ASCALE)
    sch_mul = float(128.0 / ASCALE)
    sch_add = float(127 * 128 - 5.5)

    with tile.TileContext(nc) as tc, nc.allow_low_precision(
            reason="bf16/fp8 operands feed f32-psum matmuls"):
        with (
            tc.tile_pool(name="inp", bufs=1) as inp,
            tc.tile_pool(name="expp", bufs=8) as expp,
            tc.tile_pool(name="otnp", bufs=4) as otnp,
            tc.tile_pool(name="otp", bufs=1) as otp,
            tc.tile_pool(name="recp", bufs=4) as recp,
            tc.tile_pool(name="ysbp", bufs=4) as ysbp,
            tc.tile_pool(name="ps_big", bufs=3, space="PSUM") as ps_big,
            tc.tile_pool(name="ps_av", bufs=2, space="PSUM") as ps_av,
        ):
            # ------------- inputs (chunked; critical path first) ----------
            qtil = inp.tile([33, 2 * HPC, S], fp8, tag="qtil")
            ktil = inp.tile([33, 2 * HPC, S], fp8, tag="ktil")
            v1 = inp.tile([128, NKT, HPC, 65], bf16, tag="v1")
            wf = inp.tile([128, 2, D], bf16, tag="wf")
            iden = inp.tile([128, 128], bf16, tag="iden")

            # one combined first-chunk DMA (k+q of head-pair 0, first 512
            # keys/queries) so a single transfer gates the first matmul
            qk0 = inp.tile([33, 8, 512], fp8, tag="qk0")
            nc.sync.dma_start(out=qk0, in_=qk0_d[:, :, :])
            nc.sync.dma_start(out=v1[:, 0:4, :, :], in_=v1_d[:, 0:4, :, :])
            nc.sync.dma_start(out=ktil[:, 0:4, 0:512], in_=kt_d[:, 0:4, 0:512])
            nc.sync.dma_start(out=ktil[:, 0:4, 512:2048],
                              in_=kt_d[:, 0:4, 512:2048])
            nc.sync.dma_start(out=qtil[:, 0:4, 0:512], in_=qt_d[:, 0:4, 0:512])
            nc.sync.dma_start(out=iden, in_=id_d[:, :])
            for c in range(1, 8):
                nc.sync.dma_start(out=v1[:, 2 * c:2 * c + 2, :, :],
                                  in_=v1_d[:, 2 * c:2 * c + 2, :, :])
            nc.sync.dma_start(out=ktil[:, 4:8, :], in_=kt_d[:, 4:8, :])
            nc.sync.dma_start(out=qtil[:, 4:8, 0:512], in_=qt_d[:, 4:8, 0:512])
            nc.sync.dma_start(out=qtil[:, :, 512:2048],
                              in_=qt_d[:, :, 512:2048])
            nc.sync.dma_start(out=wf, in_=wf_d[:, :, :])

            oT = []
            for p in range(2):
                t = otp.tile([128, S], bf16, tag=f"oT{p}", name=f"oT{p}")
                oT.append(t)

            # tail work (normalize/transpose/evac/fc) is deferred into the
            # NEXT pair's k-tile stream via a pop queue so it never blocks
            # the in-order Act/DVE queues at a pair boundary
            deferred = []

            def emit_norm(av, rec, otn, h2, on_act=False):
                nc.vector.reciprocal(out=rec, in_=av[:, 64:512:128])
                if on_act:
                    for qs in range(4):
                        nc.scalar.mul(otn[:, 128 * qs + 64 * h2:
                                          128 * qs + 64 * h2 + 64],
                                      av[:, 128 * qs:128 * qs + 64],
                                      rec[:, qs:qs + 1])
                else:
                    # all 4 qs blocks in one op: (av x 1.0) * recip
                    # broadcast along a zero-stride free dim
                    blocks = av[:, :].rearrange("p (a b) -> p a b",
                                                b=128)[:, :, 0:64]
                    ob = otn[:, :].rearrange("p (a b) -> p a b",
                                             b=128)[:, :, 64 * h2:
                                                    64 * h2 + 64]
                    nc.vector.scalar_tensor_tensor(
                        out=ob, in0=blocks, scalar=1.0,
                        in1=rec[:, :].broadcast_to([128, 4, 64]),
                        op0=MULT, op1=MULT)

            def emit_transpose(otn, p, q0, qb):
                tp = ps_big.tile([128, 1024], f32, tag="sc",
                                 name=f"tp{qb}_{p}")[:, 0:256].bitcast(bf16)
                for qs in range(4):
                    nc.tensor.matmul(tp[:, 128 * qs:128 * qs + 128],
                                     otn[:, 128 * qs:128 * qs + 128],
                                     iden[:, :], is_transpose=True,
                                     start=qs == 0, stop=True,
                                     skip_group_check=True)
                return tp

            def emit_ot_evac(tp, p, q0):
                nc.scalar.copy(oT[p][:, q0:q0 + 512], tp)

            def emit_fc_mm(stt):
                # two cb halves: a single matmul may not write across a
                # 2KB psum bank boundary, so the [128,1024] slot is filled
                # by two [128,512] accumulation groups
                yp = ps_big.tile([128, 1024], f32, tag="sc",
                                 name=f"y{stt}")
                for cb in range(2):
                    cs = slice(512 * cb, 512 * cb + 512)
                    nc.tensor.matmul(yp[:, cs],
                                     oT[0][:, 128 * stt:128 * stt + 128],
                                     wf[:, 0, cs], start=True, stop=False,
                                     skip_group_check=True)
                    nc.tensor.matmul(yp[:, cs],
                                     oT[1][:, 128 * stt:128 * stt + 128],
                                     wf[:, 1, cs], start=False, stop=True,
                                     skip_group_check=True)
                return yp

            def emit_fc_out(yp, stt, on_act=False):
                ysb = ysbp.tile([128, 1024], bf16, tag="ysb",
                                name=f"ysb{stt}")
                if on_act:
                    nc.scalar.copy(ysb, yp)
                else:
                    nc.vector.tensor_copy(ysb, yp)
                nc.sync.dma_start(
                    out=y_d[128 * stt:128 * stt + 128, :], in_=ysb)

            # prewarm the Exp activation table during the input DMAs
            warm = recp.tile([128, 1], f32, tag="warm")
            nc.vector.memset(warm, 0.0)
            nc.scalar.activation(out=warm, in_=warm, func=EXP,
                                 scale=act_scale)

            # single flat software-pipelined stream over all 8 (qb, p)
            # pairs: scores/exp run 2 k-tile steps ahead of the AV
            # consumers, crossing pair boundaries without a bubble
            pairs = [(qb, p) for qb in range(NQB) for p in range(2)]
            NP = len(pairs)
            pend = []
            avt = {}
            for gi in range(NP * NKT + 3):
                while deferred and deferred[0][0] <= gi:
                    deferred.pop(0)[1]()
                if gi < NP * NKT:
                    pi, kt = divmod(gi, NKT)
                    qb, p = pairs[pi]
                    q0 = 512 * qb
                    hA, hB = 2 * p, 2 * p + 1
                    ks = slice(128 * kt, 128 * kt + 128)
                    sc = ps_big.tile([128, 1024], f32, tag="sc",
                                    name=f"sc{qb}_{p}_{kt}")
                    if pi == 0 and kt < 4:
                        kA, qA = qk0[:, 0:2, ks], qk0[:, 4:6, :]
                        kB, qB = qk0[:, 2:4, ks], qk0[:, 6:8, :]
                    else:
                        kA = ktil[:, 2 * hA:2 * hA + 2, ks]
                        qA = qtil[:, 2 * hA:2 * hA + 2, q0:q0 + 512]
                        kB = ktil[:, 2 * hB:2 * hB + 2, ks]
                        qB = qtil[:, 2 * hB:2 * hB + 2, q0:q0 + 512]
                    nc.tensor.matmul(sc[:, 0:512], kA, qA,
                                     start=True, stop=True, perf_mode=DR)
                    nc.tensor.matmul(sc[:, 512:1024], kB, qB,
                                     start=True, stop=True, perf_mode=DR)
                    ex = expp.tile([128, 1024], bf16, tag="ex",
                                   name=f"ex{qb}_{p}_{kt}")
                    if kt not in _DVE_KTS:
                        nc.scalar.activation(out=ex, in_=sc, func=EXP,
                                             scale=act_scale)
                    else:
                        nc.vector.tensor_scalar(
                            out=ex.bitcast(i16), in0=sc,
                            scalar1=sch_mul, scalar2=sch_add,
                            op0=MULT, op1=ADD)
                    pend.append((pi, kt, ex))
                if gi >= 3:
                    api, akt, aex = pend[gi - 3]
                    aqb, ap = pairs[api]
                    if akt == 0:
                        # one 2KB bank per accumulator; qs blocks padded to
                        # 128 floats so only the qs==0 matmul carries
                        # start=True (a start marks the whole 2KB
                        # zero-region, so interleaved groups must share a
                        # single start per bank)
                        avt[api] = (
                            ps_av.tile([128, 512], f32, tag="av",
                                       name=f"avA{api}"),
                            ps_av.tile([128, 512], f32, tag="av",
                                       name=f"avB{api}"))
                    st = akt == 0
                    sp = akt == NKT - 1
                    for h2, hh in ((0, 2 * ap), (1, 2 * ap + 1)):
                        av = avt[api][h2]
                        for qs in range(4):
                            nc.tensor.matmul(
                                av[:, 128 * qs:128 * qs + 65],
                                aex[:, 512 * h2 + 128 * qs:
                                    512 * h2 + 128 * qs + 128],
                                v1[:, akt, hh, :],
                                start=st and qs == 0, stop=sp,
                                skip_group_check=True)
                    if sp:
                        # pair finished: queue its tail work
                        avA, avB = avt.pop(api)
                        aq0 = 512 * aqb
                        otn = otnp.tile([128, 512], bf16, tag="otn",
                                        name=f"otn{api}")
                        recA = recp.tile([128, 4], f32, tag="rec",
                                         name=f"recA{api}")
                        recB = recp.tile([128, 4], f32, tag="rec",
                                         name=f"recB{api}")
                        last = api == NP - 1
                        deferred.append((gi,
                            lambda av=avA, r=recA, o=otn, la=last:
                                emit_norm(av, r, o, 0, on_act=la)))
                        deferred.append((gi + 1,
                            lambda av=avB, r=recB, o=otn:
                                emit_norm(av, r, o, 1, on_act=False)))
                        tpbox = []
                        deferred.append((gi + 2,
                            lambda o=otn, p=ap, q0=aq0, qb=aqb, b=tpbox:
                                b.append(emit_transpose(o, p, q0, qb))))
                        deferred.append((gi + 6,
                            lambda p=ap, q0=aq0, b=tpbox:
                                emit_ot_evac(b[0], p, q0)))
                        if ap == 1:
                            for i4 in range(4):
                                oa = i4 % 3 == 0
                                ybox = []
                                deferred.append((gi + 4 + 4 * i4,
                                    lambda stt=4 * aqb + i4, b=ybox:
                                        b.append(emit_fc_mm(stt))))
                                deferred.append((gi + 6 + 4 * i4,
                                    lambda stt=4 * aqb + i4, oa=oa, b=ybox:
                                        emit_fc_out(b[0], stt, on_act=oa)))
            while deferred:
                deferred.pop(0)[1]()

    nc.compile()
    return nc


def _prep(query, key, value, Wq, bq, Wk, bk, Wv, bv, Wfc, bfc):
    """Host-side sharding / layout prep. Returns (in_maps, bfc_eff)."""
    import ml_dtypes

    query = np.asarray(query, dtype=np.float32)
    key = np.asarray(key, dtype=np.float32)
    value = np.asarray(value, dtype=np.float32)
    Wq = np.asarray(Wq, np.float32); bq = np.asarray(bq, np.float32)
    Wk = np.asarray(Wk, np.float32); bk = np.asarray(bk, np.float32)
    Wv = np.asarray(Wv, np.float32); bv = np.asarray(bv, np.float32)
    Wfc = np.asarray(Wfc, np.float32); bfc = np.asarray(bfc, np.float32)

    s_hd = np.float32(1.0 / np.sqrt(HD))
    # fold Wq into the K side: score*log2e = q . (M k) + w . k   (per head)
    M = (Wq.T @ Wk) * (s_hd * LOG2E * ASCALE)          # [d, e]
    w_row = (bq @ Wk) * (s_hd * LOG2E * ASCALE)        # [e]

    # fold Wv / bv into fc
    A = np.empty((D, D), np.float32)
    bfc_eff = bfc.copy()
    for h in range(HEAD):
        Wfc_h = Wfc[:, HD * h:HD * h + HD]
        A[:, HD * h:HD * h + HD] = Wfc_h @ Wv
        bfc_eff += Wfc_h @ bv
    At = np.ascontiguousarray(A.T)                     # [ch, c]

    iden = np.eye(128, dtype=ml_dtypes.bfloat16)

    in_maps = []
    for core in range(N_CORES):
        b, hg = core // 4, core % 4
        ch0 = CH * hg
        qtil = np.zeros((33, 2 * HPC, S), np.float32)
        ktil = np.zeros((33, 2 * HPC, S), np.float32)
        v1 = np.empty((128, NKT, HPC, 65), np.float32)
        for h in range(HPC):
            qh = query[b][:, ch0 + HD * h:ch0 + HD * h + HD]   # [S, 64]
            kh = key[b][:, ch0 + HD * h:ch0 + HD * h + HD]
            kt = kh @ M.T                                      # [S, 64]
            qtil[0:32, 2 * h, :] = qh[:, 0:32].T
            qtil[0:32, 2 * h + 1, :] = qh[:, 32:64].T
            qtil[32, 2 * h, :] = 1.0
            ktil[0:32, 2 * h, :] = kt[:, 0:32].T
            ktil[0:32, 2 * h + 1, :] = kt[:, 32:64].T
            ktil[32, 2 * h, :] = kh @ w_row
            vh = value[b][:, ch0 + HD * h:ch0 + HD * h + HD]
            v1[:, :, h, 0:64] = vh.reshape(NKT, 128, HD).transpose(1, 0, 2)
            v1[:, :, h, 64] = 1.0
        wfc = np.empty((128, 2, D), np.float32)
        wfc[:, 0, :] = At[ch0:ch0 + 128]
        wfc[:, 1, :] = At[ch0 + 128:ch0 + 256]
        qk0 = np.concatenate([ktil[:, 0:4, 0:512], qtil[:, 0:4, 0:512]],
                             axis=1)
        in_maps.append({
            "qk0": qk0.astype(ml_dtypes.float8_e4m3).view(np.uint8),
            "qtil": qtil.astype(ml_dtypes.float8_e4m3).view(np.uint8),
            "ktil": ktil.astype(ml_dtypes.float8_e4m3).view(np.uint8),
            "v1": v1.astype(ml_dtypes.bfloat16).view(np.uint16),
            "wf": wfc.astype(ml_dtypes.bfloat16).view(np.uint16),
            "iden": iden.view(np.uint16),
        })
    return in_maps, bfc_eff


def _run_once(inputs):
    global LAST_RESULTS
    from concourse.bass_utils import run_bass_kernel_spmd

    if "nc" not in _CACHE:
        _CACHE["nc"] = _build()
    nc = _CACHE["nc"]

    in_maps, bfc_eff = _prep(**inputs)
    res = run_bass_kernel_spmd(nc, in_maps, core_ids=list(range(N_CORES)))
    LAST_RESULTS = res

    import ml_dtypes
    out = np.empty((B, S, D), np.float32)
    for b in range(B):
        acc = np.zeros((S, D), np.float32)
        for hg in range(4):
            yv = res.results[4 * b + hg]["y"]
            if yv.dtype != np.float32:
                yv = yv.view(ml_dtypes.bfloat16).astype(np.float32)
            acc += yv
        out[b] = acc + bfc_eff
    return out


def kernel(**inputs) -> np.ndarray:
    last_exc = None
    for attempt in range(3):
        try:
            out = _run_once(inputs)
            amax = float(np.abs(out).max())
            if np.isfinite(out).all() and 1e-6 < amax < 1e3:
                return out
            raise RuntimeError(f"implausible kernel output (absmax={amax})")
        except Exception as e:  # noqa: BLE001 - retry transient HW failures
            last_exc = e
            _CACHE.pop("nc", None)
            _CACHE["nonce"] = attempt + 1
    raise last_exc


# revision 49
# speedup vs baseline: 1.0560x; 1.0069x over previous
"""Multi-head attention (B=2, S=2048, D=1024, H=16) on 8 Trainium2 NeuronCores.

Sharding: core i handles batch b = i//4 and head-group hg = i%4 (4 heads).
The fc layer is sharded over its contraction dim (each core emits a partial
y summed on the host); Wv/bv are folded into Wfc/bfc on the host (exact).

Algorithm (v3):
  - Wq is folded into the K side on the host: score = q . k~ with
    k~ = (Wq^T Wk / sqrt(hd)) k + bias-row, so no on-device projections.
    log2(e) and a power-of-two fp8 range scale are folded in as well, so
    the device computes t = score*log2e and exponentiates as 2^t.
  - scores: fp8e4m3 DoubleRow matmuls (contraction 2x33 packs the 64 head
    channels + bias row), out [128 keys, 512 q] psum f32 at 0.5 cycles/row.
  - exp: split per k-tile across the only two PSUM-capable elementwise
    engines (GPSIMD cannot access PSUM on TRN2): ScalarE exact exp->bf16
    (scale=ln2/ascale) for 10/16 tiles, DVE Schraudolph for 6/16 (one
    tensor_scalar mult+add writing int16 exponent-bits through a
    bf16-tile bitcast, ~3% rel err; end-to-end rel err ~1.2e-2).
  - AV transposed: exp tile is the stationary operand [128 keys, 128 q],
    V (bf16, with a ones column for the denominator) streams as moving
    [128, 65] -> out [q, 64ch|den] psum, 65 cycles per k-tile: softmax
    normalization becomes a per-partition reciprocal+scalar-multiply.
  - oTn [128 q, 64A|64B] bf16 pairs are PE-transposed (identity moving)
    to [128 ch, 128 q] and the fc runs K=256 over two bf16 matmuls per
    512-wide psum bank; y is evacuated to SBUF (DVE) and DMA'd out.

Schedule: one flat software-pipelined stream over the 8 (q-window, pair)
units; scores/exp run 3 k-tile steps ahead of the AV consumers. All psum
transients (score tiles, transpose psum, fc psum) share one 3-slot
[128,1024] rotation (6 banks) + 2 banks of AV accumulators; a start=True
matmul zeroes its whole 2KB bank, so interleaved accumulation groups in
one bank carry exactly one start. Tail work (normalize, transpose, oT
evac, fc) is deferred into later stream steps via a not-before-gi queue
so it never blocks the in-order ScalarE/DVE queues at a pair boundary.
"""


import sys

import numpy as np

if "/opt/trn_rl_repo" not in sys.path:
    sys.path.insert(0, "/opt/trn_rl_repo")

HEAD = 16
B, S, D = 2, 2048, 1024
HD = 64
HPC = 4          # heads per core
CH = HPC * HD    # channels per core
N_CORES = 8
NKT = S // 128   # k tiles
NQB = S // 512   # q windows
LOG2E = 1.4426950408889634
ASCALE = 2.0     # fp8 range scale folded into k~; undone in the exp scale

_CACHE = {}
LAST_RESULTS = None


# Per-pair k-tile positions whose exp runs on DVE (Schraudolph); the rest
# run exact exp on ScalarE. 6/16 on DVE globally balances ScalarE's 1038ns
# exp + oT-evac load against DVE's 1193ns exp + y-evac/norm/recip load,
# and keeping kt 0/1 on ScalarE lets DVE run the previous pair's deferred
# normalization immediately at each pair boundary.
# GPSIMD can't touch PSUM on TRN2, so only these two engines qualify.
_DVE_KTS = {1, 4, 6, 9, 11, 14}


def _build():
    import concourse.tile as tile
    from concourse import bacc, mybir

    f32 = mybir.dt.float32
    bf16 = mybir.dt.bfloat16
    fp8 = mybir.dt.float8e4
    i16 = mybir.dt.int16
    EXP = mybir.ActivationFunctionType.Exp
    DR = mybir.MatmulPerfMode.DoubleRow
    MULT = mybir.AluOpType.mult
    ADD = mybir.AluOpType.add

    nc = bacc.Bacc("TRN2", target_bir_lowering=False, debug=False,
                   num_devices=N_CORES)

    # unused internal tensor whose name varies per retry: changes the BIR
    # content hash so a retry never reuses a possibly-corrupt cached NEFF
    nonce = _CACHE.get("nonce", 0)
    if nonce:
        nc.dram_tensor(f"retry_nonce_{nonce}", [1, 1], mybir.dt.float32)

    qt_d = nc.dram_tensor("qtil", [33, 2 * HPC, S], fp8, kind="ExternalInput")
    kt_d = nc.dram_tensor("ktil", [33, 2 * HPC, S], fp8, kind="ExternalInput")
    v1_d = nc.dram_tensor("v1", [128, NKT, HPC, 65], bf16,
                          kind="ExternalInput")
    wf_d = nc.dram_tensor("wf", [128, 2, D], bf16, kind="ExternalInput")
    id_d = nc.dram_tensor("iden", [128, 128], bf16, kind="ExternalInput")
    qk0_d = nc.dram_tensor("qk0", [33, 8, 512], fp8, kind="ExternalInput")
    y_d = nc.dram_tensor("y", [S, D], bf16, kind="ExternalOutput")

    act_scale = float(np.log(2.0) / ASCALE)
    sch_mul = float(128.0 / ASCALE)
    sch_add = float(127 * 128 - 5.5)

    with tile.TileContext(nc) as tc, nc.allow_low_precision(
            reason="bf16/fp8 operands feed f32-psum matmuls"):
        with (
            tc.tile_pool(name="inp", bufs=1) as inp,
            tc.tile_pool(name="expp", bufs=8) as expp,
            tc.tile_pool(name="otnp", bufs=4) as otnp,
            tc.tile_pool(name="otp", bufs=1) as otp,
            tc.tile_pool(name="recp", bufs=4) as recp,
            tc.tile_pool(name="ysbp", bufs=4) as ysbp,
            tc.tile_pool(name="ps_big", bufs=3, space="PSUM") as ps_big,
            tc.tile_pool(name="ps_av", bufs=2, space="PSUM") as ps_av,
        ):
            # ------------- inputs (chunked; critical path first) ----------
            qtil = inp.tile([33, 2 * HPC, S], fp8, tag="qtil")
            ktil = inp.tile([33, 2 * HPC, S], fp8, tag="ktil")
            v1 = inp.tile([128, NKT, HPC, 65], bf16, tag="v1")
            wf = inp.tile([128, 2, D], bf16, tag="wf")
            iden = inp.tile([128, 128], bf16, tag="iden")

            # one combined first-chunk DMA (k+q of head-pair 0, first 512
            # keys/queries) so a single transfer gates the first matmul
            qk0 = inp.tile([33, 8, 512], fp8, tag="qk0")
            nc.sync.dma_start(out=qk0, in_=qk0_d[:, :, :])
            nc.sync.dma_start(out=v1[:, 0:4, :, :], in_=v1_d[:, 0:4, :, :])
            nc.sync.dma_start(out=ktil[:, 0:4, 0:512], in_=kt_d[:, 0:4, 0:512])
            nc.sync.dma_start(out=ktil[:, 0:4, 512:2048],
                              in_=kt_d[:, 0:4, 512:2048])
            nc.sync.dma_start(out=qtil[:, 0:4, 0:512], in_=qt_d[:, 0:4, 0:512])
            nc.sync.dma_start(out=iden, in_=id_d[:, :])
            for c in range(1, 8):
                nc.sync.dma_start(out=v1[:, 2 * c:2 * c + 2, :, :],
                                  in_=v1_d[:, 2 * c:2 * c + 2, :, :])
            nc.sync.dma_start(out=ktil[:, 4:8, :], in_=kt_d[:, 4:8, :])
            nc.sync.dma_start(out=qtil[:, 4:8, 0:512], in_=qt_d[:, 4:8, 0:512])
            nc.sync.dma_start(out=qtil[:, :, 512:2048],
                              in_=qt_d[:, :, 512:2048])
            nc.sync.dma_start(out=wf, in_=wf_d[:, :, :])

            oT = []
            for p in range(2):
                t = otp.tile([128, S], bf16, tag=f"oT{p}", name=f"oT{p}")
                oT.append(t)

            # tail work (normalize/transpose/evac/fc) is deferred into the
            # NEXT pair's k-tile stream via a pop queue so it never blocks
            # the in-order Act/DVE queues at a pair boundary
            deferred = []

            def emit_norm(av, rec, otn, h2, on_act=False):
                nc.vector.reciprocal(out=rec, in_=av[:, 64:512:128])
                if on_act:
                    for qs in range(4):
                        nc.scalar.mul(otn[:, 128 * qs + 64 * h2:
                                          128 * qs + 64 * h2 + 64],
                                      av[:, 128 * qs:128 * qs + 64],
                                      rec[:, qs:qs + 1])
                else:
                    # all 4 qs blocks in one op: (av x 1.0) * recip
                    # broadcast along a zero-stride free dim
                    blocks = av[:, :].rearrange("p (a b) -> p a b",
                                                b=128)[:, :, 0:64]
                    ob = otn[:, :].rearrange("p (a b) -> p a b",
                                             b=128)[:, :, 64 * h2:
                                                    64 * h2 + 64]
                    nc.vector.scalar_tensor_tensor(
                        out=ob, in0=blocks, scalar=1.0,
                        in1=rec[:, :].broadcast_to([128, 4, 64]),
                        op0=MULT, op1=MULT)

            def emit_transpose(otn, p, q0, qb):
                tp = ps_big.tile([128, 1024], f32, tag="sc",
                                 name=f"tp{qb}_{p}")[:, 0:256].bitcast(bf16)
                for qs in range(4):
                    nc.tensor.matmul(tp[:, 128 * qs:128 * qs + 128],
                                     otn[:, 128 * qs:128 * qs + 128],
                                     iden[:, :], is_transpose=True,
                                     start=qs == 0, stop=True,
                                     skip_group_check=True)
                return tp

            def emit_ot_evac(tp, p, q0):
                nc.scalar.copy(oT[p][:, q0:q0 + 512], tp)

            def emit_fc_mm(stt):
                # two cb halves: a single matmul may not write across a
                # 2KB psum bank boundary, so the [128,1024] slot is filled
                # by two [128,512] accumulation groups
                yp = ps_big.tile([128, 1024], f32, tag="sc",
                                 name=f"y{stt}")
                for cb in range(2):
                    cs = slice(512 * cb, 512 * cb + 512)
                    nc.tensor.matmul(yp[:, cs],
                                     oT[0][:, 128 * stt:128 * stt + 128],
                                     wf[:, 0, cs], start=True, stop=False,
                                     skip_group_check=True)
                    nc.tensor.matmul(yp[:, cs],
                                     oT[1][:, 128 * stt:128 * stt + 128],
                                     wf[:, 1, cs], start=False, stop=True,
                                     skip_group_check=True)
                return yp

            def emit_fc_out(yp, stt, on_act=False):
                ysb = ysbp.tile([128, 1024], bf16, tag="ysb",
                                name=f"ysb{stt}")
                if on_act:
                    nc.scalar.copy(ysb, yp)
                else:
                    nc.vector.tensor_copy(ysb, yp)
                nc.sync.dma_start(
                    out=y_d[128 * stt:128 * stt + 128, :], in_=ysb)

            # prewarm the Exp activation table during the input DMAs
            warm = recp.tile([128, 1], f32, tag="warm")
            nc.vector.memset(warm, 0.0)
            nc.scalar.activation(out=warm, in_=warm, func=EXP,
                                 scale=act_scale)

            # single flat software-pipelined stream over all 8 (qb, p)
            # pairs: scores/exp run 2 k-tile steps ahead of the AV
            # consumers, crossing pair boundaries without a bubble
            pairs = [(qb, p) for qb in range(NQB) for p in range(2)]
            NP = len(pairs)
            pend = []
            avt = {}
            for gi in range(NP * NKT + 3):
                while deferred and deferred[0][0] <= gi:
                    deferred.pop(0)[1]()
                if gi < NP * NKT:
                    pi, kt = divmod(gi, NKT)
                    qb, p = pairs[pi]
                    q0 = 512 * qb
                    hA, hB = 2 * p, 2 * p + 1
                    ks = slice(128 * kt, 128 * kt + 128)
                    sc = ps_big.tile([128, 1024], f32, tag="sc",
                                    name=f"sc{qb}_{p}_{kt}")
                    if pi == 0 and kt < 4:
                        kA, qA = qk0[:, 0:2, ks], qk0[:, 4:6, :]
                        kB, qB = qk0[:, 2:4, ks], qk0[:, 6:8, :]
                    else:
                        kA = ktil[:, 2 * hA:2 * hA + 2, ks]
                        qA = qtil[:, 2 * hA:2 * hA + 2, q0:q0 + 512]
                        kB = ktil[:, 2 * hB:2 * hB + 2, ks]
                        qB = qtil[:, 2 * hB:2 * hB + 2, q0:q0 + 512]
                    nc.tensor.matmul(sc[:, 0:512], kA, qA,
                                     start=True, stop=True, perf_mode=DR)
                    nc.tensor.matmul(sc[:, 512:1024], kB, qB,
                                     start=True, stop=True, perf_mode=DR)
                    ex = expp.tile([128, 1024], bf16, tag="ex",
                                   name=f"ex{qb}_{p}_{kt}")
                    if kt not in _DVE_KTS:
                        nc.scalar.activation(out=ex, in_=sc, func=EXP,
                                             scale=act_scale)
                    else:
                        nc.vector.tensor_scalar(
                            out=ex.bitcast(i16), in0=sc,
                            scalar1=sch_mul, scalar2=sch_add,
                            op0=MULT, op1=ADD)
                    pend.append((pi, kt, ex))
                if gi >= 3:
                    api, akt, aex = pend[gi - 3]
                    aqb, ap = pairs[api]
                    if akt == 0:
                        # one 2KB bank per accumulator; qs blocks padded to
                        # 128 floats so only the qs==0 matmul carries
                        # start=True (a start marks the whole 2KB
                        # zero-region, so interleaved groups must share a
                        # single start per bank)
                        avt[api] = (
                            ps_av.tile([128, 512], f32, tag="av",
                                       name=f"avA{api}"),
                            ps_av.tile([128, 512], f32, tag="av",
                                       name=f"avB{api}"))
                    st = akt == 0
                    sp = akt == NKT - 1
                    for h2, hh in ((0, 2 * ap), (1, 2 * ap + 1)):
                        av = avt[api][h2]
                        for qs in range(4):
                            nc.tensor.matmul(
                                av[:, 128 * qs:128 * qs + 65],
                                aex[:, 512 * h2 + 128 * qs:
                                    512 * h2 + 128 * qs + 128],
                                v1[:, akt, hh, :],
                                start=st and qs == 0, stop=sp,
                                skip_group_check=True)
                    if sp:
                        # pair finished: queue its tail work
                        avA, avB = avt.pop(api)
                        aq0 = 512 * aqb
                        otn = otnp.tile([128, 512], bf16, tag="otn",
                                        name=f"otn{api}")
                        recA = recp.tile([128, 4], f32, tag="rec",
                                         name=f"recA{api}")
                        recB = recp.tile([128, 4], f32, tag="rec",
                                         name=f"recB{api}")
                        last = api == NP - 1
                        deferred.append((gi,
                            lambda av=avA, r=recA, o=otn:
                                emit_norm(av, r, o, 0, on_act=False)))
                        deferred.append((gi + 1,
                            lambda av=avB, r=recB, o=otn:
                                emit_norm(av, r, o, 1, on_act=False)))
                        tpbox = []
                        deferred.append((gi + 2,
                            lambda o=otn, p=ap, q0=aq0, qb=aqb, b=tpbox:
                                b.append(emit_transpose(o, p, q0, qb))))
                        deferred.append((gi + 6,
                            lambda p=ap, q0=aq0, b=tpbox:
                                emit_ot_evac(b[0], p, q0)))
                        if ap == 1:
                            for i4 in range(4):
                                oa = i4 % 3 == 0
                                ybox = []
                                deferred.append((gi + 4 + 4 * i4,
                                    lambda stt=4 * aqb + i4, b=ybox:
                                        b.append(emit_fc_mm(stt))))
                                deferred.append((gi + 6 + 4 * i4,
                                    lambda stt=4 * aqb + i4, oa=oa, b=ybox:
                                        emit_fc_out(b[0], stt, on_act=oa)))
            while deferred:
                deferred.pop(0)[1]()

    nc.compile()
    return nc


def _prep(query, key, value, Wq, bq, Wk, bk, Wv, bv, Wfc, bfc):
    """Host-side sharding / layout prep. Returns (in_maps, bfc_eff)."""
    import ml_dtypes

    query = np.asarray(query, dtype=np.float32)
    key = np.asarray(key, dtype=np.float32)
    value = np.asarray(value, dtype=np.float32)
    Wq = np.asarray(Wq, np.float32); bq = np.asarray(bq, np.float32)
    Wk = np.asarray(Wk, np.float32); bk = np.asarray(bk, np.float32)
    Wv = np.asarray(Wv, np.float32); bv = np.asarray(bv, np.float32)
    Wfc = np.asarray(Wfc, np.float32); bfc = np.asarray(bfc, np.float32)

    s_hd = np.float32(1.0 / np.sqrt(HD))
    # fold Wq into the K side: score*log2e = q . (M k) + w . k   (per head)
    M = (Wq.T @ Wk) * (s_hd * LOG2E * ASCALE)          # [d, e]
    w_row = (bq @ Wk) * (s_hd * LOG2E * ASCALE)        # [e]

    # fold Wv / bv into fc
    A = np.empty((D, D), np.float32)
    bfc_eff = bfc.copy()
    for h in range(HEAD):
        Wfc_h = Wfc[:, HD * h:HD * h + HD]
        A[:, HD * h:HD * h + HD] = Wfc_h @ Wv
        bfc_eff += Wfc_h @ bv
    At = np.ascontiguousarray(A.T)                     # [ch, c]

    iden = np.eye(128, dtype=ml_dtypes.bfloat16)

    in_maps = []
    for core in range(N_CORES):
        b, hg = core // 4, core % 4
        ch0 = CH * hg
        qtil = np.zeros((33, 2 * HPC, S), np.float32)
        ktil = np.zeros((33, 2 * HPC, S), np.float32)
        v1 = np.empty((128, NKT, HPC, 65), np.float32)
        for h in range(HPC):
            qh = query[b][:, ch0 + HD * h:ch0 + HD * h + HD]   # [S, 64]
            kh = key[b][:, ch0 + HD * h:ch0 + HD * h + HD]
            kt = kh @ M.T                                      # [S, 64]
            qtil[0:32, 2 * h, :] = qh[:, 0:32].T
            qtil[0:32, 2 * h + 1, :] = qh[:, 32:64].T
            qtil[32, 2 * h, :] = 1.0
            ktil[0:32, 2 * h, :] = kt[:, 0:32].T
            ktil[0:32, 2 * h + 1, :] = kt[:, 32:64].T
            ktil[32, 2 * h, :] = kh @ w_row
            vh = value[b][:, ch0 + HD * h:ch0 + HD * h + HD]
            v1[:, :, h, 0:64] = vh.reshape(NKT, 128, HD).transpose(1, 0, 2)
            v1[:, :, h, 64] = 1.0
        wfc = np.empty((128, 2, D), np.float32)
        wfc[:, 0, :] = At[ch0:ch0 + 128]
        wfc[:, 1, :] = At[ch0 + 128:ch0 + 256]
        qk0 = np.concatenate([ktil[:, 0:4, 0:512], qtil[:, 0:4, 0:512]],
                             axis=1)
        in_maps.append({
            "qk0": qk0.astype(ml_dtypes.float8_e4m3).view(np.uint8),
            "qtil": qtil.astype(ml_dtypes.float8_e4m3).view(np.uint8),
            "ktil": ktil.astype(ml_dtypes.float8_e4m3).view(np.uint8),
            "v1": v1.astype(ml_dtypes.bfloat16).view(np.uint16),
            "wf": wfc.astype(ml_dtypes.bfloat16).view(np.uint16),
            "iden": iden.view(np.uint16),
        })
    return in_maps, bfc_eff


def _run_once(inputs):
    global LAST_RESULTS
    from concourse.bass_utils import run_bass_kernel_spmd

    if "nc" not in _CACHE:
        _CACHE["nc"] = _build()
    nc = _CACHE["nc"]

    in_maps, bfc_eff = _prep(**inputs)
    res = run_bass_kernel_spmd(nc, in_maps, core_ids=list(range(N_CORES)))
    LAST_RESULTS = res

    import ml_dtypes
    out = np.empty((B, S, D), np.float32)
    for b in range(B):
        acc = np.zeros((S, D), np.float32)
        for hg in range(4):
            yv = res.results[4 * b + hg]["y"]
            if yv.dtype != np.float32:
                yv = yv.view(ml_dtypes.bfloat16).astype(np.float32)
            acc += yv
        out[b] = acc + bfc_eff
    return out


def kernel(**inputs) -> np.ndarray:
    last_exc = None
    for attempt in range(3):
        try:
            out = _run_once(inputs)
            amax = float(np.abs(out).max())
            if np.isfinite(out).all() and 1e-6 < amax < 1e3:
                return out
            raise RuntimeError(f"implausible kernel output (absmax={amax})")
        except Exception as e:  # noqa: BLE001 - retry transient HW failures
            last_exc = e
            _CACHE.pop("nc", None)
            _CACHE["nonce"] = attempt + 1
    raise last_exc
